# revision 1
# baseline (speedup 1.0000x reference)
"""Trainium2 Bass kernel for nn_AugmentedAttentionHead.

Per batch element b:
    q = LN(x_b @ Wq); k = LN(x_b @ Wk); v = x_b @ Wv
    S = q k^T / sqrt(D);  P = softmax(S, axis=-1)
    sigma = sigmoid(q @ Wsig + bsig)[:, 0]   (per key)
    alpha = softplus(q @ Walp + balp)        (per query)
    out_b = (P * sigma[None, :] * alpha[:, None]) @ v

Device restructuring:
  * out_b = diag(alpha / rowsum(E)) @ E @ diag(sigma) @ v, E = exp(S/sqrt(D)).
    sigma folds into v rows; alpha and the softmax normalization fold into one
    per-row output scale. LayerNormed q,k bound the logits (|S| < ~7 for these
    inputs), so exp() without max-subtraction is safe in fp32.
  * The gate pre-activations (q_n @ [Wsig0|Walp]) are algebraically pushed
    through the LayerNorm: gate_j = rstd*((x@Wqg)_j - mean*csum_j) + badd_j
    with Wqg = Wq @ wg and csum = colsum(wg) precomputed on host. x@Wqg rides
    as two extra columns of the v projection - no extra matmuls.
  * Attention phase computes S^T blocks (keys on partitions) so exp() output
    IS the PV lhsT - no on-chip transposes of the attention matrix. The E
    rowsum rides as a ones-column appended to v.
  * rstd = exp(-0.5*ln(var+eps)), sigmoid/softplus built from exp/ln: the ACT
    engine then needs only the {exp, ln, copy} LUT set (this image's walrus
    has no softplus/sigmoid/sqrt table co-resident with exp).

Sharding: data-parallel over batch B=8 across the 8 NeuronCores (one batch
element per core, weights replicated). No collectives.

Matmuls run as float32r (full PE rate at free-dim>=256, ~tf32 precision;
measured |err|_max/|out|_max ~ 3e-4) accumulating fp32 in PSUM. Set
MM_DT="f32" for exact-fp32 (4x slower) matmuls.
"""

import numpy as np


def _ensure_concourse():
    try:
        import concourse.bass  # noqa: F401
        return
    except ImportError:
        pass
    import sys

    for p in ("/opt/trn_rl_repo", "/root/.axon_site/_ro/trn_rl_repo"):
        if p not in sys.path:
            sys.path.insert(0, p)
    import concourse.bass  # noqa: F401


B, T, D = 8, 2048, 768
PT = 128          # partition tile
NT = T // PT      # 16 row tiles
ND = D // PT      # 6 contraction subtiles
NST = 4           # phase-2 super-tiles
STQ = T // NST    # 512 query rows per super-tile
EPS = 1e-5
SCALE = 1.0 / np.sqrt(np.float32(D))
HALF = D // 2     # 384

MM_DT = "f32r"    # "f32r" (fast) or "f32" (exact, 4x slower)


def _install_ldw_opt():
    """Re-enable walrus's LDWEIGHTS dedup (consecutive matmuls sharing a
    stationary tile skip the reload). Verified bit-identical rel-err on HW."""
    import concourse.bass_utils as bu

    if getattr(bu, "_ldw_opt_installed", False):
        return
    real_run = bu.run_command

    def run_patched(cmd, cwd=None):
        cmd = [
            "--enable-ldw-opt=true" if c == "--enable-ldw-opt=false" else c
            for c in cmd
        ]
        return real_run(cmd, cwd=cwd)

    bu.run_command = run_patched
    bu._ldw_opt_installed = True


def _install_tile_drain_fix():
    """walrus CoreV3 in this image allows only ONE sync-wait per CTRL-class
    (Drain/NoOp) instruction, but TileContext's exit drain accumulates one
    wait per logical processor. Split the waits across single-wait NoOps."""
    import concourse.tile as tile
    import concourse.mybir as mybir
    from concourse.vector_clock import ScopedClock

    if getattr(tile.TileContext, "_drain_fix_installed", False):
        return

    def _patched(self, tick_clock, wait_clock):
        nc = self.nc
        drain_inst = nc.sync.drain()
        wait_clock.add_sem_waits(
            drain_inst.ins, ScopedClock({None: tick_clock.global_clock})
        )
        si = drain_inst.ins.sync_info
        waits = list(si.on_wait or []) if si else []
        if len(waits) > 1:
            si.on_wait = waits[:1]
            for i in range(1, len(waits)):
                nop = nc.sync.nop(nofuse=True, hint="drain_wait_overflow")
                nop.ins.sync_info = mybir.SyncInfo(
                    on_wait=waits[i : i + 1], on_update=[]
                )
        nc.all_engine_barrier()
        assert self.sems is not None
        popped = nc._tile_sem_poison_stack.pop()
        assert popped is self._sem_poison
        nc.clear_and_free_semaphores(list(self.sems.allocated().values()))
        nc.all_engine_barrier()

    tile.TileContext._drain_and_barrier = _patched
    tile.TileContext._drain_fix_installed = True


def _split_excess_waits(nc, mybir, limit=1):
    """walrus CoreV3 here accepts only `limit` sync-waits per instruction.
    Move excess waits onto single-wait NoOps inserted immediately before the
    over-limit instruction on the same engine (waiting earlier on the same
    engine is order-preserving and safe)."""
    blocks = nc.m.functions[0].blocks
    snaps = [(b, list(b.instructions)) for b in blocks]
    plans = []
    for b, insts in snaps:
        plan = []
        for i, inst in enumerate(insts):
            si = inst.sync_info
            waits = list(si.on_wait) if si and si.on_wait else []
            if len(waits) > limit:
                plan.append((i, waits[: len(waits) - limit]))
                si.on_wait = waits[len(waits) - limit :]
        plans.append(plan)
    rebuilt = []
    for (b, insts), plan in zip(snaps, plans):
        plan_by_idx = dict(plan)
        out = []
        for i, inst in enumerate(insts):
            for w in plan_by_idx.get(i, ()):
                nop = nc.engines[inst.engine].nop(nofuse=True, hint="wait_split")
                nop.ins.sync_info = mybir.SyncInfo(on_wait=[w], on_update=[])
                out.append(nop.ins)
            out.append(inst)
        rebuilt.append((b, out))
    # Assign EVERY block (even plan-free ones): nop() auto-appends to the live
    # current bb, so unassigned blocks would keep duplicate stray nops.
    for b, out in rebuilt:
        b.instructions = out


def build_nc(mm_dt_name=MM_DT, ln_gain=False, gate_adds=(0.0, 0.0),
             csum=(0.0, 0.0)):
    """Build the single-core Bass program (SPMD across 8 cores).

    Inputs : x [T, D], wq/wk [D, D], wv [D, D+2] (= [Wv | Wq@wg/SCALE] with
             wg = [Wsig[:,0], Walp[:,0]], gains pre-applied when ln_gain).
             Optional: qg/qb/kg/kb [PT, D] broadcast LN gain/bias (q's bias
             pre-scaled by SCALE).
    Output : out [T, D]
    gate_adds: per-gate additive consts (bias terms), baked into the program.
    csum   : colsum(wg)/SCALE consts, baked into the program.
    """
    _ensure_concourse()
    import concourse.bass as bass
    import concourse.tile as tile
    import concourse.mybir as mybir
    from concourse.masks import make_identity

    _install_tile_drain_fix()
    _install_ldw_opt()

    f32 = mybir.dt.float32
    mm_dt = {"f32r": mybir.dt.float32r, "f32": mybir.dt.float32}[mm_dt_name]
    AF = mybir.ActivationFunctionType
    Alu = mybir.AluOpType

    DV = D + 2  # v projection width incl. gate columns

    nc = bass.Bass()
    xT_d = nc.dram_tensor("xT", [NT, PT, ND, PT], mm_dt, kind="ExternalInput")
    wq_d = nc.dram_tensor("wq", [D, D], mm_dt, kind="ExternalInput")
    wk_d = nc.dram_tensor("wk", [D, D], mm_dt, kind="ExternalInput")
    wv_d = nc.dram_tensor("wv", [D, DV], mm_dt, kind="ExternalInput")
    if ln_gain:
        qg_d = nc.dram_tensor("qg", [PT, D], f32, kind="ExternalInput")
        qb_d = nc.dram_tensor("qb", [PT, D], f32, kind="ExternalInput")
        kg_d = nc.dram_tensor("kg", [PT, D], f32, kind="ExternalInput")
        kb_d = nc.dram_tensor("kb", [PT, D], f32, kind="ExternalInput")
    out_d = nc.dram_tensor("out", [T, D], f32, kind="ExternalOutput")
    qnT_d = nc.dram_tensor("qnT_tmp", [NT, PT, ND, PT], mm_dt)  # internal scratch

    with tile.TileContext(nc) as tc:
        with (
            tc.tile_pool(name="persist", bufs=1) as persist,
            tc.tile_pool(name="consts", bufs=1) as consts,
        ):
            ident_f = consts.tile([PT, PT], f32, tag="identf")
            make_identity(nc, ident_f)
            ident = consts.tile([PT, PT], mm_dt, tag="ident")
            nc.vector.tensor_copy(out=ident, in_=ident_f)
            eps_t = consts.tile([PT, 1], f32, tag="eps")
            nc.vector.memset(eps_t, EPS)
            # rstd = exp(-0.5*ln(var+eps) [+ ln(SCALE) for q's fold])
            lnsc_t = consts.tile([PT, 1], f32, tag="lnsc")
            nc.vector.memset(lnsc_t, float(np.log(SCALE)))
            ones16 = consts.tile([PT, NT], f32, tag="ones16")
            nc.vector.memset(ones16, 1.0)

            knT_res = persist.tile([PT, ND, T], mm_dt, tag="knT")
            v_res = persist.tile([PT, NT, D + 2], mm_dt, tag="v")  # +ones cols
            # (two ones columns: f32r matmul needs an even moving free-dim)
            alpha_res = persist.tile([PT, NT], f32, tag="alpha")
            # ones column of v (rowsum rider); memset can't write f32r
            nc.vector.tensor_copy(out=v_res[:, :, D], in_=ones16)
            nc.vector.tensor_copy(out=v_res[:, :, D + 1], in_=ones16)

            if ln_gain:
                qg_t = consts.tile([PT, D], f32, tag="qg")
                qb_t = consts.tile([PT, D], f32, tag="qb")
                kg_t = consts.tile([PT, D], f32, tag="kg")
                kb_t = consts.tile([PT, D], f32, tag="kb")
                nc.sync.dma_start(out=qg_t, in_=qg_d[:, :])
                nc.sync.dma_start(out=qb_t, in_=qb_d[:, :])
                nc.sync.dma_start(out=kg_t, in_=kg_d[:, :])
                nc.sync.dma_start(out=kb_t, in_=kb_d[:, :])

            # ---------------- Phase 1: projections + LN + gates ----------
            with (
                tc.tile_pool(name="weights", bufs=1) as wpool,
                tc.tile_pool(name="ph1", bufs=2) as ph1,
                tc.tile_pool(name="ph1x", bufs=3) as ph1x,
                tc.tile_pool(name="ph1s", bufs=3) as ph1s,
                tc.tile_pool(name="xt_ps", bufs=2, space="PSUM") as xt_ps,
                tc.tile_pool(name="proj_ps", bufs=3, space="PSUM") as proj_ps,
            ):
                wq_t = wpool.tile([PT, ND, D], mm_dt, tag="wq")
                wk_t = wpool.tile([PT, ND, D], mm_dt, tag="wk")
                wv_t = wpool.tile([PT, ND, DV], mm_dt, tag="wv")
                nc.sync.dma_start(
                    out=wq_t, in_=wq_d.rearrange("(o p) f -> p o f", p=PT)
                )
                nc.sync.dma_start(
                    out=wk_t, in_=wk_d.rearrange("(o p) f -> p o f", p=PT)
                )
                nc.sync.dma_start(
                    out=wv_t, in_=wv_d.rearrange("(o p) f -> p o f", p=PT)
                )

                for t in range(NT):
                    r0 = t * PT
                    xT = ph1x.tile([PT, ND, PT], mm_dt, tag="xT")
                    nc.sync.dma_start(out=xT, in_=xT_d[t])

                    # six psum accumulators (2 tags x 3 bufs rotate across
                    # q/k/v and tiles); dt-outer so the stationary xT tile is
                    # reused by 6 consecutive matmuls
                    qa = proj_ps.tile([PT, HALF], f32, tag="pa")
                    qb = proj_ps.tile([PT, HALF], f32, tag="pb")
                    ka = proj_ps.tile([PT, HALF], f32, tag="pa")
                    kb = proj_ps.tile([PT, HALF], f32, tag="pb")
                    va = proj_ps.tile([PT, HALF], f32, tag="pa")
                    vb = proj_ps.tile([PT, DV - HALF], f32, tag="pb")
                    targets = (
                        (qa, wq_t, 0, HALF), (qb, wq_t, HALF, D),
                        (ka, wk_t, 0, HALF), (kb, wk_t, HALF, D),
                        (va, wv_t, 0, HALF), (vb, wv_t, HALF, DV),
                    )
                    for dt in range(ND):
                        for ps, w_t, c0, c1 in targets:
                            nc.tensor.matmul(
                                ps,
                                lhsT=xT[:, dt, :],
                                rhs=w_t[:, dt, c0:c1],
                                start=(dt == 0),
                                stop=(dt == ND - 1),
                            )

                    # fast PSUM release: stats read psum, ACT copies evacuate
                    # q/k (and v straight into its resident slab) so the next
                    # tile's matmuls aren't gated on the LayerNorm chain
                    gcol = D - HALF  # local index of global col D within vb
                    qsb = ph1.tile([PT, D], mm_dt, tag="qsb")
                    ksb = ph1.tile([PT, D], mm_dt, tag="ksb")
                    gsb = ph1s.tile([PT, 2], f32, tag="gsb")
                    stats_q = ph1s.tile([PT, 2, 6], f32, tag="stq")
                    stats_k = ph1s.tile([PT, 2, 6], f32, tag="stk")
                    nc.vector.bn_stats(out=stats_q[:, 0, :], in_=qa)
                    nc.vector.bn_stats(out=stats_q[:, 1, :], in_=qb)
                    nc.vector.bn_stats(out=stats_k[:, 0, :], in_=ka)
                    nc.vector.bn_stats(out=stats_k[:, 1, :], in_=kb)
                    nc.scalar.activation(out=qsb[:, 0:HALF], in_=qa, func=AF.Copy)
                    nc.scalar.activation(out=qsb[:, HALF:D], in_=qb, func=AF.Copy)
                    nc.scalar.activation(out=ksb[:, 0:HALF], in_=ka, func=AF.Copy)
                    nc.scalar.activation(out=ksb[:, HALF:D], in_=kb, func=AF.Copy)
                    nc.vector.tensor_copy(out=v_res[:, t, 0:HALF], in_=va)
                    nc.vector.tensor_copy(
                        out=v_res[:, t, HALF:D], in_=vb[:, 0:gcol]
                    )
                    nc.vector.tensor_copy(
                        out=gsb, in_=vb[:, gcol : gcol + 2]
                    )

                    # LayerNorm applied in place on the SBUF copies; q folds
                    # SCALE into rstd
                    def layernorm(stats, sb, fold_scale, g_t, b_t, tagn):
                        mv = ph1s.tile([PT, 2], f32, tag=f"mv{tagn}")
                        nc.vector.bn_aggr(out=mv, in_=stats)
                        lnv = ph1s.tile([PT, 1], f32, tag=f"ln{tagn}")
                        nc.scalar.activation(
                            out=lnv, in_=mv[:, 1:2], func=AF.Ln, bias=eps_t
                        )
                        r = ph1s.tile([PT, 1], f32, tag=f"r{tagn}")
                        nc.scalar.activation(
                            out=r, in_=lnv, func=AF.Exp, scale=-0.5,
                            bias=lnsc_t if fold_scale else 0.0,
                        )
                        for c in range(2):
                            sl = slice(c * HALF, (c + 1) * HALF)
                            nc.vector.tensor_scalar(
                                out=sb[:, sl],
                                in0=sb[:, sl],
                                scalar1=mv[:, 0:1],
                                scalar2=r,
                                op0=Alu.subtract,
                                op1=Alu.mult,
                            )
                        if g_t is not None:
                            nc.vector.tensor_mul(out=sb, in0=sb, in1=g_t)
                        if b_t is not None:
                            nc.vector.tensor_add(out=sb, in0=sb, in1=b_t)
                        return mv, r

                    mv_q, r_q = layernorm(
                        stats_q, qsb, True,
                        qg_t if ln_gain else None, qb_t if ln_gain else None, "q",
                    )
                    layernorm(
                        stats_k, ksb, False,
                        kg_t if ln_gain else None, kb_t if ln_gain else None, "k",
                    )

                    # gates: gate_j = r_q*(raw_j - mean_q*csum_j) + gate_adds_j
                    # (r_q carries 1/SCALE via the fold; csum was pre-divided)
                    sig_t = ph1s.tile([PT, 1], f32, tag="sig")
                    alp_t = ph1s.tile([PT, 1], f32, tag="alp")
                    for j, gout in ((0, sig_t), (1, alp_t)):
                        mc = ph1s.tile([PT, 1], f32, tag=f"mc{j}")
                        nc.vector.tensor_scalar_mul(
                            out=mc, in0=mv_q[:, 0:1], scalar1=float(csum[j])
                        )
                        nc.vector.tensor_scalar(
                            out=gout,
                            in0=gsb[:, j : j + 1],
                            scalar1=mc,
                            scalar2=r_q,
                            op0=Alu.subtract,
                            op1=Alu.mult,
                        )
                        if gate_adds[j] != 0.0:
                            nc.vector.tensor_scalar_add(
                                out=gout, in0=gout, scalar1=float(gate_adds[j])
                            )
                    # sigma = 1/(1+exp(-g0)); alpha = ln(1+exp(g1))
                    nc.scalar.activation(
                        out=sig_t, in_=sig_t, func=AF.Exp, scale=-1.0
                    )
                    nc.vector.tensor_scalar_add(out=sig_t, in0=sig_t, scalar1=1.0)
                    nc.vector.reciprocal(out=sig_t, in_=sig_t)
                    nc.scalar.activation(out=alp_t, in_=alp_t, func=AF.Exp)
                    nc.vector.tensor_scalar_add(out=alp_t, in0=alp_t, scalar1=1.0)
                    nc.scalar.activation(
                        out=alpha_res[:, t : t + 1], in_=alp_t, func=AF.Ln
                    )

                    # sigma fold on the resident v rows (in place)
                    nc.vector.tensor_scalar_mul(
                        out=v_res[:, t, 0:D], in0=v_res[:, t, 0:D], scalar1=sig_t
                    )

                    # transpose qn -> qnT (to DRAM); kn -> knT_res
                    qnT = ph1.tile([PT, ND, PT], mm_dt, tag="qnT")
                    for dt in range(ND):
                        tp = xt_ps.tile([PT, PT], mm_dt, tag="xtp")
                        nc.tensor.transpose(
                            tp, qsb[:, dt * PT : (dt + 1) * PT], ident
                        )
                        if dt % 2 == 0:
                            nc.vector.tensor_copy(out=qnT[:, dt, :], in_=tp)
                        else:
                            nc.scalar.activation(
                                out=qnT[:, dt, :], in_=tp, func=AF.Copy
                            )
                    for dt in range(ND):
                        tp = xt_ps.tile([PT, PT], mm_dt, tag="xtp")
                        nc.tensor.transpose(
                            tp, ksb[:, dt * PT : (dt + 1) * PT], ident
                        )
                        if dt % 2 == 0:
                            nc.vector.tensor_copy(
                                out=knT_res[:, dt, r0 : r0 + PT], in_=tp
                            )
                        else:
                            nc.scalar.activation(
                                out=knT_res[:, dt, r0 : r0 + PT], in_=tp,
                                func=AF.Copy,
                            )
                    nc.sync.dma_start(out=qnT_d[t], in_=qnT)

            # ---------------- Phase 2: attention (S^T blocks) -------------
            with (
                tc.tile_pool(name="ph2", bufs=2) as ph2,
                tc.tile_pool(name="ph2s", bufs=3) as ph2s,
                tc.tile_pool(name="s_ps", bufs=3, space="PSUM") as s_ps,
                tc.tile_pool(name="pv_ps", bufs=2, space="PSUM") as pv_ps,
            ):
                for st in range(NST):
                    q0 = st * STQ
                    qnT_st = ph2.tile([PT, ND, STQ], mm_dt, tag="qnT2")
                    for qs in range(NST):
                        nc.sync.dma_start(
                            out=qnT_st[:, :, qs * PT : (qs + 1) * PT],
                            in_=qnT_d[st * NST + qs],
                        )
                    # S^T blocks: [keys(128) x STQ], exp() lands directly in
                    # the PV lhsT slab
                    ET = ph2.tile([PT, NT, STQ], mm_dt, tag="ET")
                    for kbi in range(NT):
                        sp = s_ps.tile([PT, STQ], f32, tag="s")
                        for dt in range(ND):
                            nc.tensor.matmul(
                                sp,
                                lhsT=knT_res[:, dt, kbi * PT : (kbi + 1) * PT],
                                rhs=qnT_st[:, dt, :],
                                start=(dt == 0),
                                stop=(dt == ND - 1),
                            )
                        nc.scalar.activation(
                            out=ET[:, kbi, :], in_=sp, func=AF.Exp
                        )

                    for qs in range(NST):
                        t = st * NST + qs
                        r0 = t * PT
                        qsl = slice(qs * PT, (qs + 1) * PT)
                        pp0 = pv_ps.tile([PT, HALF], f32, tag="pv0")
                        pp1 = pv_ps.tile([PT, D + 2 - HALF], f32, tag="pv1")
                        for kbi in range(NT):
                            nc.tensor.matmul(
                                pp0,
                                lhsT=ET[:, kbi, qsl],
                                rhs=v_res[:, kbi, 0:HALF],
                                start=(kbi == 0),
                                stop=(kbi == NT - 1),
                            )
                            nc.tensor.matmul(
                                pp1,
                                lhsT=ET[:, kbi, qsl],
                                rhs=v_res[:, kbi, HALF : D + 2],
                                start=(kbi == 0),
                                stop=(kbi == NT - 1),
                            )
                        # rowsum is pp1's last column; fold alpha & normalize
                        rsc = ph2s.tile([PT, 1], f32, tag="rsc")
                        nc.vector.reciprocal(out=rsc, in_=pp1[:, D - HALF : D - HALF + 1])
                        rowscale = ph2s.tile([PT, 1], f32, tag="rssc")
                        nc.vector.tensor_mul(
                            out=rowscale, in0=rsc, in1=alpha_res[:, t : t + 1]
                        )
                        o_sb = ph2.tile([PT, D], f32, tag="o")
                        nc.vector.tensor_scalar_mul(
                            out=o_sb[:, 0:HALF], in0=pp0, scalar1=rowscale
                        )
                        nc.vector.tensor_scalar_mul(
                            out=o_sb[:, HALF:D],
                            in0=pp1[:, 0 : D - HALF],
                            scalar1=rowscale,
                        )
                        nc.sync.dma_start(out=out_d[r0 : r0 + PT, :], in_=o_sb)

    _split_excess_waits(nc, mybir)
    return nc


_NC_CACHE = {}


def _get_nc(key):
    if key not in _NC_CACHE:
        _NC_CACHE[key] = build_nc(*key)
    return _NC_CACHE[key]


def make_in_maps(inputs):
    """Host-side prep: per-core input maps + build key."""
    x = np.asarray(inputs["x"], dtype=np.float32)
    Wq = np.asarray(inputs["Wq"], dtype=np.float64)
    Wk = np.asarray(inputs["Wk"], dtype=np.float32)
    Wv = np.asarray(inputs["Wv"], dtype=np.float32)
    qn_g = np.asarray(inputs["qn_g"], dtype=np.float64)
    qn_b = np.asarray(inputs["qn_b"], dtype=np.float64)
    kn_g = np.asarray(inputs["kn_g"], dtype=np.float32)
    kn_b = np.asarray(inputs["kn_b"], dtype=np.float32)
    Wsig = np.asarray(inputs["Wsig"], dtype=np.float64)
    bsig = np.asarray(inputs["bsig"], dtype=np.float64)
    Walp = np.asarray(inputs["Walp"], dtype=np.float64)
    balp = np.asarray(inputs["balp"], dtype=np.float64)

    ln_gain = not (
        np.all(qn_g == 1) and np.all(qn_b == 0)
        and np.all(kn_g == 1) and np.all(kn_b == 0)
    )

    # gate columns: wg = [Wsig[:,0], Walp[:,0]] with LN gain applied; the
    # matmul term rides the v projection as x @ (Wq @ wg) / SCALE (q's rstd
    # carries SCALE). Bias terms (b @ wg + gate bias) are additive consts.
    wg = np.stack([Wsig[:, 0], Walp[:, 0]], axis=1)  # [D, 2] float64
    wg_g = qn_g[:, None] * wg
    Wqg = (Wq @ wg_g) / SCALE                        # [D, 2]
    csum = wg_g.sum(axis=0) / SCALE                  # [2]
    badd = qn_b @ wg                                 # [2]
    gate_adds = (float(badd[0] + bsig[0]), float(badd[1] + balp[0]))

    wv_ext = np.concatenate(
        [Wv.astype(np.float64), Wqg], axis=1
    ).astype(np.float32)  # [D, D+2]

    key = (MM_DT, ln_gain, gate_adds, (float(csum[0]), float(csum[1])))

    base = {
        "wq": np.ascontiguousarray(Wq.astype(np.float32)),
        "wk": np.ascontiguousarray(Wk),
        "wv": np.ascontiguousarray(wv_ext),
    }
    if ln_gain:
        base["qg"] = np.broadcast_to(qn_g.astype(np.float32), (PT, D)).copy()
        base["qb"] = np.broadcast_to(
            (qn_b * SCALE).astype(np.float32), (PT, D)
        ).copy()
        base["kg"] = np.broadcast_to(kn_g, (PT, D)).copy()
        base["kb"] = np.broadcast_to(kn_b, (PT, D)).copy()

    # blocked transpose: xT[t, p, o, f] = x[b, t*PT+f, o*PT+p]
    xTb = np.ascontiguousarray(
        x.reshape(B, NT, PT, ND, PT).transpose(0, 1, 4, 3, 2)
    )
    in_maps = [dict(base, xT=xTb[b]) for b in range(B)]
    return in_maps, key


def run(inputs, trace=False, mm_dt=None):
    _ensure_concourse()
    import time
    from concourse.bass_utils import run_bass_kernel_spmd

    in_maps, key = make_in_maps(inputs)
    if mm_dt is not None:
        key = (mm_dt,) + key[1:]
    nc = _get_nc(key)
    res = None
    for attempt in range(3):
        try:
            res = run_bass_kernel_spmd(
                nc, in_maps, core_ids=list(range(B)), trace=trace
            )
            break
        except Exception:
            # transient "accelerator device unrecoverable" wedges heal after
            # a cooldown; retry rather than failing the whole call
            if attempt == 2:
                raise
            time.sleep(75)
    out = np.stack([res.results[b]["out"] for b in range(B)]).astype(np.float32)
    return out, res


def kernel(**inputs) -> np.ndarray:
    out, _ = run(inputs)
    return out



# revision 7
# speedup vs baseline: 1.1224x; 1.1224x over previous
"""Trainium2 Bass kernel for nn_AugmentedAttentionHead.

Per batch element b:
    q = LN(x_b @ Wq); k = LN(x_b @ Wk); v = x_b @ Wv
    S = q k^T / sqrt(D);  P = softmax(S, axis=-1)
    sigma = sigmoid(q @ Wsig + bsig)[:, 0]   (per key)
    alpha = softplus(q @ Walp + balp)        (per query)
    out_b = (P * sigma[None, :] * alpha[:, None]) @ v

Device restructuring:
  * out_b = diag(alpha / rowsum(E)) @ E @ diag(sigma) @ v, E = exp(S/sqrt(D)).
    sigma folds into v rows; alpha and the softmax normalization fold into one
    per-row output scale. LayerNormed q,k bound the logits (|S| < ~6.6 for
    these inputs), so exp() without max-subtraction is safe.
  * The gate pre-activations (q_n @ [Wsig0|Walp]) are algebraically pushed
    through the LayerNorm: gate_j = rstd*((x@Wqg)_j - mean*csum_j) + badd_j
    with Wqg = Wq @ wg and csum = colsum(wg) precomputed on host. x@Wqg rides
    as two extra columns of the v projection - no extra matmuls.
  * Attention phase computes S^T blocks (keys on partitions) so exp() output
    IS the PV lhsT - no on-chip transposes of the attention matrix. The E
    rowsum rides as a ones-column appended to v.
  * rstd = exp(-0.5*ln(var+eps)), sigmoid/softplus built from exp/ln: the ACT
    engine then needs only the {exp, ln, copy} LUT set.
  * Everything runs in bf16 (same PE rate as f32r on TRN2, half the DMA and
    SBUF): x/W stream in as bf16, qn/kn/E/sigma*v are stored bf16, PSUM
    accumulation stays fp32. fp8 DoubleRow (2x PE) was tried and REJECTED:
    with absmax(out)~0.62 a single e4m3 rounding anywhere (qk / E / v) alone
    produces ~2.2-2.6e-2 max rel err, over the 2e-2 gate; error-compensated
    fp8 needs 2x the streams = parity with bf16.
  * bf16 also fits qnT resident in SBUF (no DRAM roundtrip between phases)
    and makes PE transposes 1.0 cyc/row (vs 1.5 f32r).
  * Input DMAs are issued need-ordered and fine-grained (per contraction
    subtile for the first x tile, per half for weights): with one bulk DMA
    per tensor the first matmul's operands share HBM bandwidth with 13MB of
    later-needed data and PE sat idle ~33us at kernel start.

Sharding: data-parallel over batch B=8 across the 8 NeuronCores (one batch
element per core, weights replicated). No collectives.
"""

import numpy as np


def _ensure_concourse():
    try:
        import concourse.bass  # noqa: F401
        return
    except ImportError:
        pass
    import sys

    for p in ("/opt/trn_rl_repo", "/root/.axon_site/_ro/trn_rl_repo"):
        if p not in sys.path:
            sys.path.insert(0, p)
    import concourse.bass  # noqa: F401


B, T, D = 8, 2048, 768
PT = 128          # partition tile
NT = T // PT      # 16 row tiles
ND = D // PT      # 6 contraction subtiles
NST = 4           # phase-2 super-tiles
STQ = T // NST    # 512 query rows per super-tile
EPS = 1e-5
SCALE = 1.0 / np.sqrt(np.float32(D))
HALF = D // 2     # 384

MM_DT = "bf16"    # matmul dtype: "bf16" (fast) or "f32" (exact, 4x slower)


def _install_ldw_opt():
    """Re-enable walrus's LDWEIGHTS dedup (consecutive matmuls sharing a
    stationary tile skip the reload). Verified bit-identical rel-err on HW."""
    import concourse.bass_utils as bu

    if getattr(bu, "_ldw_opt_installed", False):
        return
    real_run = bu.run_command

    def run_patched(cmd, cwd=None):
        cmd = [
            "--enable-ldw-opt=true" if c == "--enable-ldw-opt=false" else c
            for c in cmd
        ]
        return real_run(cmd, cwd=cwd)

    bu.run_command = run_patched
    bu._ldw_opt_installed = True


def _install_tile_drain_fix():
    """walrus CoreV3 in this image allows only ONE sync-wait per CTRL-class
    (Drain/NoOp) instruction, but TileContext's exit drain accumulates one
    wait per logical processor. Split the waits across single-wait NoOps."""
    import concourse.tile as tile
    import concourse.mybir as mybir
    from concourse.vector_clock import ScopedClock

    if getattr(tile.TileContext, "_drain_fix_installed", False):
        return

    def _patched(self, tick_clock, wait_clock):
        nc = self.nc
        drain_inst = nc.sync.drain()
        wait_clock.add_sem_waits(
            drain_inst.ins, ScopedClock({None: tick_clock.global_clock})
        )
        si = drain_inst.ins.sync_info
        waits = list(si.on_wait or []) if si else []
        if len(waits) > 1:
            si.on_wait = waits[:1]
            for i in range(1, len(waits)):
                nop = nc.sync.nop(nofuse=True, hint="drain_wait_overflow")
                nop.ins.sync_info = mybir.SyncInfo(
                    on_wait=waits[i : i + 1], on_update=[]
                )
        nc.all_engine_barrier()
        assert self.sems is not None
        popped = nc._tile_sem_poison_stack.pop()
        assert popped is self._sem_poison
        nc.clear_and_free_semaphores(list(self.sems.allocated().values()))
        nc.all_engine_barrier()

    tile.TileContext._drain_and_barrier = _patched
    tile.TileContext._drain_fix_installed = True


def _split_excess_waits(nc, mybir, limit=1):
    """walrus CoreV3 here accepts only `limit` sync-waits per instruction.
    Move excess waits onto single-wait NoOps inserted immediately before the
    over-limit instruction on the same engine (waiting earlier on the same
    engine is order-preserving and safe)."""
    blocks = nc.m.functions[0].blocks
    snaps = [(b, list(b.instructions)) for b in blocks]
    plans = []
    for b, insts in snaps:
        plan = []
        for i, inst in enumerate(insts):
            si = inst.sync_info
            waits = list(si.on_wait) if si and si.on_wait else []
            if len(waits) > limit:
                plan.append((i, waits[: len(waits) - limit]))
                si.on_wait = waits[len(waits) - limit :]
        plans.append(plan)
    rebuilt = []
    for (b, insts), plan in zip(snaps, plans):
        plan_by_idx = dict(plan)
        out = []
        for i, inst in enumerate(insts):
            for w in plan_by_idx.get(i, ()):
                nop = nc.engines[inst.engine].nop(nofuse=True, hint="wait_split")
                nop.ins.sync_info = mybir.SyncInfo(on_wait=[w], on_update=[])
                out.append(nop.ins)
            out.append(inst)
        rebuilt.append((b, out))
    # Assign EVERY block (even plan-free ones): nop() auto-appends to the live
    # current bb, so unassigned blocks would keep duplicate stray nops.
    for b, out in rebuilt:
        b.instructions = out


def build_nc(mm_dt_name=MM_DT, ln_gain=False, gate_adds=(0.0, 0.0),
             csum=(0.0, 0.0)):
    """Build the single-core Bass program (SPMD across 8 cores).

    Inputs : xT [NT,PT,ND,PT] (blocked x^T), wq/wk [D, D], wv [D, D+2]
             (= [Wv | Wq@wg/SCALE] with wg = [Wsig[:,0], Walp[:,0]]).
             Optional: qg/qb/kg/kb [PT, D] broadcast LN gain/bias (q's bias
             pre-scaled by SCALE).
    Output : out [T, D]
    gate_adds: per-gate additive consts (bias terms), baked into the program.
    csum   : colsum(wg)/SCALE consts, baked into the program.
    """
    _ensure_concourse()
    import concourse.bass as bass
    import concourse.tile as tile
    import concourse.mybir as mybir
    from concourse.masks import make_identity

    _install_tile_drain_fix()
    # NOTE: walrus's LDW dedup pass (--enable-ldw-opt=true) rejects bf16
    # transpose ldweights in this image; traces show LDWEIGHTS is hidden
    # behind the previous matmul's stream anyway, so it stays disabled.

    f32 = mybir.dt.float32
    mm_dt = {"bf16": mybir.dt.bfloat16, "f32": mybir.dt.float32}[mm_dt_name]
    AF = mybir.ActivationFunctionType
    Alu = mybir.AluOpType

    DV = D + 2  # v projection width incl. gate columns

    nc = bass.Bass()
    xT_d = nc.dram_tensor("xT", [NT, PT, ND, PT], mm_dt, kind="ExternalInput")
    wq_d = nc.dram_tensor("wq", [D, D], mm_dt, kind="ExternalInput")
    wk_d = nc.dram_tensor("wk", [D, D], mm_dt, kind="ExternalInput")
    wv_d = nc.dram_tensor("wv", [D, DV], mm_dt, kind="ExternalInput")
    if ln_gain:
        qg_d = nc.dram_tensor("qg", [PT, D], f32, kind="ExternalInput")
        qb_d = nc.dram_tensor("qb", [PT, D], f32, kind="ExternalInput")
        kg_d = nc.dram_tensor("kg", [PT, D], f32, kind="ExternalInput")
        kb_d = nc.dram_tensor("kb", [PT, D], f32, kind="ExternalInput")
    out_d = nc.dram_tensor("out", [T, D], f32, kind="ExternalOutput")

    with tile.TileContext(nc) as tc:
        with (
            tc.tile_pool(name="persist", bufs=1) as persist,
            tc.tile_pool(name="consts", bufs=1) as consts,
        ):
            ident_f = consts.tile([PT, PT], f32, tag="identf")
            make_identity(nc, ident_f)
            ident = consts.tile([PT, PT], mm_dt, tag="ident")
            nc.vector.tensor_copy(out=ident, in_=ident_f)
            eps_t = consts.tile([PT, 1], f32, tag="eps")
            nc.vector.memset(eps_t, EPS)
            # rstd = exp(-0.5*ln(var+eps) [+ ln(SCALE) for q's fold])
            lnsc_t = consts.tile([PT, 1], f32, tag="lnsc")
            nc.vector.memset(lnsc_t, float(np.log(SCALE)))
            ones16 = consts.tile([PT, NT], f32, tag="ones16")
            nc.vector.memset(ones16, 1.0)

            knT_res = persist.tile([PT, ND, T], mm_dt, tag="knT")
            qnT_res = persist.tile([PT, ND, T], mm_dt, tag="qnT")
            v_res = persist.tile([PT, NT, DV], mm_dt, tag="v")  # +ones cols
            alpha_res = persist.tile([PT, NT], f32, tag="alpha")
            # rowsum rider columns (exact 1.0 in bf16)
            nc.vector.tensor_copy(out=v_res[:, :, D], in_=ones16)
            nc.vector.tensor_copy(out=v_res[:, :, D + 1], in_=ones16)

            if ln_gain:
                qg_t = consts.tile([PT, D], f32, tag="qg")
                qb_t = consts.tile([PT, D], f32, tag="qb")
                kg_t = consts.tile([PT, D], f32, tag="kg")
                kb_t = consts.tile([PT, D], f32, tag="kb")
                nc.sync.dma_start(out=qg_t, in_=qg_d[:, :])
                nc.sync.dma_start(out=qb_t, in_=qb_d[:, :])
                nc.sync.dma_start(out=kg_t, in_=kg_d[:, :])
                nc.sync.dma_start(out=kb_t, in_=kb_d[:, :])

            # ---------------- Phase 1: projections + LN + gates ----------
            with (
                tc.tile_pool(name="weights", bufs=1) as wpool,
                tc.tile_pool(name="ph1", bufs=2) as ph1,
                tc.tile_pool(name="ph1x", bufs=3) as ph1x,
                tc.tile_pool(name="ph1s", bufs=3) as ph1s,
                tc.tile_pool(name="xt_ps", bufs=2, space="PSUM") as xt_ps,
                tc.tile_pool(name="proj_ps", bufs=3, space="PSUM") as proj_ps,
            ):
                wq_t = wpool.tile([PT, ND, D], mm_dt, tag="wq")
                wk_t = wpool.tile([PT, ND, D], mm_dt, tag="wk")
                wv_t = wpool.tile([PT, ND, DV], mm_dt, tag="wv")
                wq_r = wq_d.rearrange("(o p) f -> p o f", p=PT)
                wk_r = wk_d.rearrange("(o p) f -> p o f", p=PT)
                wv_r = wv_d.rearrange("(o p) f -> p o f", p=PT)

                # tile 0's x lands first (fine-grained per-subtile), then the
                # weights in the order tile 0's matmuls consume them; bulk
                # tiles follow. One bulk DMA per tensor would share HBM
                # bandwidth round-robin and stall the first matmul ~30us.
                xT0 = ph1x.tile([PT, ND, PT], mm_dt, tag="xT")
                for dt in range(ND):
                    nc.sync.dma_start(out=xT0[:, dt, :], in_=xT_d[0, :, dt, :])
                for dt in range(ND):
                    nc.sync.dma_start(
                        out=wq_t[:, dt, 0:HALF], in_=wq_r[:, dt, 0:HALF]
                    )
                    nc.sync.dma_start(
                        out=wq_t[:, dt, HALF:D], in_=wq_r[:, dt, HALF:D]
                    )
                    nc.sync.dma_start(
                        out=wk_t[:, dt, 0:HALF], in_=wk_r[:, dt, 0:HALF]
                    )
                    nc.sync.dma_start(
                        out=wk_t[:, dt, HALF:D], in_=wk_r[:, dt, HALF:D]
                    )
                    nc.sync.dma_start(
                        out=wv_t[:, dt, 0:HALF], in_=wv_r[:, dt, 0:HALF]
                    )
                    nc.sync.dma_start(
                        out=wv_t[:, dt, HALF:DV], in_=wv_r[:, dt, HALF:DV]
                    )

                for t in range(NT):
                    r0 = t * PT
                    if t == 0:
                        xT = xT0
                    else:
                        xT = ph1x.tile([PT, ND, PT], mm_dt, tag="xT")
                        nc.sync.dma_start(out=xT, in_=xT_d[t])

                    # six psum accumulators (2 tags x 3 bufs rotate across
                    # q/k/v and tiles); dt-outer so the stationary xT tile is
                    # reused by 6 consecutive matmuls
                    qa = proj_ps.tile([PT, HALF], f32, tag="pa")
                    qb = proj_ps.tile([PT, HALF], f32, tag="pb")
                    ka = proj_ps.tile([PT, HALF], f32, tag="pa")
                    kb = proj_ps.tile([PT, HALF], f32, tag="pb")
                    va = proj_ps.tile([PT, HALF], f32, tag="pa")
                    vb = proj_ps.tile([PT, DV - HALF], f32, tag="pb")
                    targets = (
                        (qa, wq_t, 0, HALF), (qb, wq_t, HALF, D),
                        (ka, wk_t, 0, HALF), (kb, wk_t, HALF, D),
                        (va, wv_t, 0, HALF), (vb, wv_t, HALF, DV),
                    )
                    for dt in range(ND):
                        for ps, w_t, c0, c1 in targets:
                            nc.tensor.matmul(
                                ps,
                                lhsT=xT[:, dt, :],
                                rhs=w_t[:, dt, c0:c1],
                                start=(dt == 0),
                                stop=(dt == ND - 1),
                            )

                    # fast PSUM release: stats read psum, ACT copies evacuate
                    # q/k so the next tile's matmuls aren't gated on the
                    # LayerNorm chain (v's psum is held until sigma is ready;
                    # the pa/pb buf rotation gives ~a full tile of slack)
                    gcol = D - HALF  # local index of global col D within vb
                    qsb = ph1.tile([PT, D], mm_dt, tag="qsb")
                    ksb = ph1.tile([PT, D], mm_dt, tag="ksb")
                    gsb = ph1s.tile([PT, 2], f32, tag="gsb")
                    stats_q = ph1s.tile([PT, 2, 6], f32, tag="stq")
                    stats_k = ph1s.tile([PT, 2, 6], f32, tag="stk")
                    nc.vector.bn_stats(out=stats_q[:, 0, :], in_=qa)
                    nc.vector.bn_stats(out=stats_q[:, 1, :], in_=qb)
                    nc.vector.bn_stats(out=stats_k[:, 0, :], in_=ka)
                    nc.vector.bn_stats(out=stats_k[:, 1, :], in_=kb)
                    nc.scalar.activation(out=qsb[:, 0:HALF], in_=qa, func=AF.Copy)
                    nc.scalar.activation(out=qsb[:, HALF:D], in_=qb, func=AF.Copy)
                    nc.scalar.activation(out=ksb[:, 0:HALF], in_=ka, func=AF.Copy)
                    nc.scalar.activation(out=ksb[:, HALF:D], in_=kb, func=AF.Copy)
                    nc.vector.tensor_copy(
                        out=gsb, in_=vb[:, gcol : gcol + 2]
                    )

                    # LayerNorm applied in place on the SBUF copies; q folds
                    # SCALE into rstd
                    def layernorm(stats, sb, fold_scale, g_t, b_t, tagn):
                        mv = ph1s.tile([PT, 2], f32, tag=f"mv{tagn}")
                        nc.vector.bn_aggr(out=mv, in_=stats)
                        lnv = ph1s.tile([PT, 1], f32, tag=f"ln{tagn}")
                        nc.scalar.activation(
                            out=lnv, in_=mv[:, 1:2], func=AF.Ln, bias=eps_t
                        )
                        r = ph1s.tile([PT, 1], f32, tag=f"r{tagn}")
                        nc.scalar.activation(
                            out=r, in_=lnv, func=AF.Exp, scale=-0.5,
                            bias=lnsc_t if fold_scale else 0.0,
                        )
                        nc.vector.tensor_scalar(
                            out=sb,
                            in0=sb,
                            scalar1=mv[:, 0:1],
                            scalar2=r,
                            op0=Alu.subtract,
                            op1=Alu.mult,
                        )
                        if g_t is not None:
                            nc.vector.tensor_mul(out=sb, in0=sb, in1=g_t)
                        if b_t is not None:
                            nc.vector.tensor_add(out=sb, in0=sb, in1=b_t)
                        return mv, r

                    mv_q, r_q = layernorm(
                        stats_q, qsb, True,
                        qg_t if ln_gain else None, qb_t if ln_gain else None, "q",
                    )
                    layernorm(
                        stats_k, ksb, False,
                        kg_t if ln_gain else None, kb_t if ln_gain else None, "k",
                    )

                    # gates: gate_j = r_q*(raw_j - mean_q*csum_j) + gate_adds_j
                    # (r_q carries 1/SCALE via the fold; csum was pre-divided)
                    sig_t = ph1s.tile([PT, 1], f32, tag="sig")
                    alp_t = ph1s.tile([PT, 1], f32, tag="alp")
                    for j, gout in ((0, sig_t), (1, alp_t)):
                        mc = ph1s.tile([PT, 1], f32, tag=f"mc{j}")
                        nc.vector.tensor_scalar_mul(
                            out=mc, in0=mv_q[:, 0:1], scalar1=float(csum[j])
                        )
                        nc.vector.tensor_scalar(
                            out=gout,
                            in0=gsb[:, j : j + 1],
                            scalar1=mc,
                            scalar2=r_q,
                            op0=Alu.subtract,
                            op1=Alu.mult,
                        )
                        if gate_adds[j] != 0.0:
                            nc.vector.tensor_scalar_add(
                                out=gout, in0=gout, scalar1=float(gate_adds[j])
                            )
                    # sigma = 1/(1+exp(-g0)); alpha = ln(1+exp(g1))
                    nc.scalar.activation(
                        out=sig_t, in_=sig_t, func=AF.Exp, scale=-1.0
                    )
                    nc.vector.tensor_scalar_add(out=sig_t, in0=sig_t, scalar1=1.0)
                    nc.vector.reciprocal(out=sig_t, in_=sig_t)
                    nc.scalar.activation(out=alp_t, in_=alp_t, func=AF.Exp)
                    nc.vector.tensor_scalar_add(out=alp_t, in0=alp_t, scalar1=1.0)
                    nc.scalar.activation(
                        out=alpha_res[:, t : t + 1], in_=alp_t, func=AF.Ln
                    )

                    # v evacuation with the sigma fold, straight to bf16 (one
                    # rounding)
                    nc.vector.tensor_scalar_mul(
                        out=v_res[:, t, 0:HALF], in0=va, scalar1=sig_t
                    )
                    nc.vector.tensor_scalar_mul(
                        out=v_res[:, t, HALF:D], in0=vb[:, 0:gcol], scalar1=sig_t
                    )

                    # transpose qn/kn (bf16, 1 cyc/row) into the resident
                    # slabs; no DRAM roundtrip
                    for dt in range(ND):
                        tp = xt_ps.tile([PT, PT], mm_dt, tag="xtp")
                        nc.tensor.transpose(
                            tp, qsb[:, dt * PT : (dt + 1) * PT], ident
                        )
                        if dt % 2 == 0:
                            nc.vector.tensor_copy(
                                out=qnT_res[:, dt, r0 : r0 + PT], in_=tp
                            )
                        else:
                            nc.scalar.activation(
                                out=qnT_res[:, dt, r0 : r0 + PT], in_=tp,
                                func=AF.Copy,
                            )
                    for dt in range(ND):
                        tp = xt_ps.tile([PT, PT], mm_dt, tag="xtp")
                        nc.tensor.transpose(
                            tp, ksb[:, dt * PT : (dt + 1) * PT], ident
                        )
                        if dt % 2 == 0:
                            nc.vector.tensor_copy(
                                out=knT_res[:, dt, r0 : r0 + PT], in_=tp
                            )
                        else:
                            nc.scalar.activation(
                                out=knT_res[:, dt, r0 : r0 + PT], in_=tp,
                                func=AF.Copy,
                            )

            # ---------------- Phase 2: attention (S^T blocks) -------------
            with (
                tc.tile_pool(name="ph2", bufs=2) as ph2,
                tc.tile_pool(name="ph2s", bufs=3) as ph2s,
                tc.tile_pool(name="s_ps", bufs=3, space="PSUM") as s_ps,
                tc.tile_pool(name="pv_ps", bufs=2, space="PSUM") as pv_ps,
            ):
                for st in range(NST):
                    q0 = st * STQ
                    # S^T blocks: [keys(128) x STQ], exp() lands directly in
                    # the PV lhsT slab
                    ET = ph2.tile([PT, NT, STQ], mm_dt, tag="ET")
                    for kbi in range(NT):
                        sp = s_ps.tile([PT, STQ], f32, tag="s")
                        for dt in range(ND):
                            nc.tensor.matmul(
                                sp,
                                lhsT=knT_res[:, dt, kbi * PT : (kbi + 1) * PT],
                                rhs=qnT_res[:, dt, q0 : q0 + STQ],
                                start=(dt == 0),
                                stop=(dt == ND - 1),
                            )
                        nc.scalar.activation(
                            out=ET[:, kbi, :], in_=sp, func=AF.Exp
                        )

                    for qs in range(NST):
                        t = st * NST + qs
                        r0 = t * PT
                        qsl = slice(qs * PT, (qs + 1) * PT)
                        pp0 = pv_ps.tile([PT, HALF], f32, tag="pv0")
                        pp1 = pv_ps.tile([PT, DV - HALF], f32, tag="pv1")
                        for kbi in range(NT):
                            nc.tensor.matmul(
                                pp0,
                                lhsT=ET[:, kbi, qsl],
                                rhs=v_res[:, kbi, 0:HALF],
                                start=(kbi == 0),
                                stop=(kbi == NT - 1),
                            )
                            nc.tensor.matmul(
                                pp1,
                                lhsT=ET[:, kbi, qsl],
                                rhs=v_res[:, kbi, HALF : DV],
                                start=(kbi == 0),
                                stop=(kbi == NT - 1),
                            )
                        # rowsum rider sits at pp1 col D-HALF; fold alpha and
                        # the softmax normalization into one per-row scale
                        rsc = ph2s.tile([PT, 1], f32, tag="rsc")
                        nc.vector.reciprocal(
                            out=rsc, in_=pp1[:, D - HALF : D - HALF + 1]
                        )
                        rowscale = ph2s.tile([PT, 1], f32, tag="rssc")
                        nc.vector.tensor_mul(
                            out=rowscale, in0=rsc, in1=alpha_res[:, t : t + 1]
                        )
                        o_sb = ph2.tile([PT, D], f32, tag="o")
                        nc.vector.tensor_scalar_mul(
                            out=o_sb[:, 0:HALF], in0=pp0, scalar1=rowscale
                        )
                        nc.vector.tensor_scalar_mul(
                            out=o_sb[:, HALF:D],
                            in0=pp1[:, 0 : D - HALF],
                            scalar1=rowscale,
                        )
                        nc.sync.dma_start(out=out_d[r0 : r0 + PT, :], in_=o_sb)

    _split_excess_waits(nc, mybir)
    return nc


_NC_CACHE = {}


def _get_nc(key):
    if key not in _NC_CACHE:
        _NC_CACHE[key] = build_nc(*key)
    return _NC_CACHE[key]


def make_in_maps(inputs):
    """Host-side prep: per-core input maps + build key."""
    import ml_dtypes

    np_mm = ml_dtypes.bfloat16 if MM_DT == "bf16" else np.float32
    x = np.asarray(inputs["x"], dtype=np.float32)
    Wq = np.asarray(inputs["Wq"], dtype=np.float64)
    Wk = np.asarray(inputs["Wk"], dtype=np.float32)
    Wv = np.asarray(inputs["Wv"], dtype=np.float32)
    qn_g = np.asarray(inputs["qn_g"], dtype=np.float64)
    qn_b = np.asarray(inputs["qn_b"], dtype=np.float64)
    kn_g = np.asarray(inputs["kn_g"], dtype=np.float32)
    kn_b = np.asarray(inputs["kn_b"], dtype=np.float32)
    Wsig = np.asarray(inputs["Wsig"], dtype=np.float64)
    bsig = np.asarray(inputs["bsig"], dtype=np.float64)
    Walp = np.asarray(inputs["Walp"], dtype=np.float64)
    balp = np.asarray(inputs["balp"], dtype=np.float64)

    ln_gain = not (
        np.all(qn_g == 1) and np.all(qn_b == 0)
        and np.all(kn_g == 1) and np.all(kn_b == 0)
    )

    # gate columns: wg = [Wsig[:,0], Walp[:,0]] with LN gain applied; the
    # matmul term rides the v projection as x @ (Wq @ wg) / SCALE (q's rstd
    # carries SCALE). Bias terms (b @ wg + gate bias) are additive consts.
    wg = np.stack([Wsig[:, 0], Walp[:, 0]], axis=1)  # [D, 2] float64
    wg_g = qn_g[:, None] * wg
    Wqg = (Wq @ wg_g) / SCALE                        # [D, 2]
    csum = wg_g.sum(axis=0) / SCALE                  # [2]
    badd = qn_b @ wg                                 # [2]
    gate_adds = (float(badd[0] + bsig[0]), float(badd[1] + balp[0]))

    wv_ext = np.concatenate(
        [Wv.astype(np.float64), Wqg], axis=1
    ).astype(np.float32)  # [D, D+2]

    key = (MM_DT, ln_gain, gate_adds, (float(csum[0]), float(csum[1])))

    base = {
        "wq": np.ascontiguousarray(Wq.astype(np_mm)),
        "wk": np.ascontiguousarray(Wk.astype(np_mm)),
        "wv": np.ascontiguousarray(wv_ext.astype(np_mm)),
    }
    if ln_gain:
        base["qg"] = np.broadcast_to(qn_g.astype(np.float32), (PT, D)).copy()
        base["qb"] = np.broadcast_to(
            (qn_b * SCALE).astype(np.float32), (PT, D)
        ).copy()
        base["kg"] = np.broadcast_to(kn_g, (PT, D)).copy()
        base["kb"] = np.broadcast_to(kn_b, (PT, D)).copy()

    # blocked transpose: xT[t, p, o, f] = x[b, t*PT+f, o*PT+p]
    xTb = np.ascontiguousarray(
        x.reshape(B, NT, PT, ND, PT).transpose(0, 1, 4, 3, 2).astype(np_mm)
    )
    in_maps = [dict(base, xT=xTb[b]) for b in range(B)]
    return in_maps, key


def run(inputs, trace=False, mm_dt=None):
    _ensure_concourse()
    import time
    from concourse.bass_utils import run_bass_kernel_spmd

    in_maps, key = make_in_maps(inputs)
    if mm_dt is not None:
        key = (mm_dt,) + key[1:]
    nc = _get_nc(key)
    res = None
    for attempt in range(3):
        try:
            res = run_bass_kernel_spmd(
                nc, in_maps, core_ids=list(range(B)), trace=trace
            )
            break
        except Exception:
            # transient "accelerator device unrecoverable" wedges heal after
            # a cooldown; retry rather than failing the whole call
            if attempt == 2:
                raise
            time.sleep(75)
    out = np.stack([res.results[b]["out"] for b in range(B)]).astype(np.float32)
    return out, res


def kernel(**inputs) -> np.ndarray:
    out, _ = run(inputs)
    return out


# revision 14
# speedup vs baseline: 1.1617x; 1.0350x over previous
"""Trainium2 Bass kernel for nn_AugmentedAttentionHead.

Per batch element b:
    q = LN(x_b @ Wq); k = LN(x_b @ Wk); v = x_b @ Wv
    S = q k^T / sqrt(D);  P = softmax(S, axis=-1)
    sigma = sigmoid(q @ Wsig + bsig)[:, 0]   (per key)
    alpha = softplus(q @ Walp + balp)        (per query)
    out_b = (P * sigma[None, :] * alpha[:, None]) @ v

Device restructuring:
  * out_b = diag(alpha / rowsum(E)) @ E @ diag(sigma) @ v, E = exp(S/sqrt(D)).
    sigma folds into v rows; alpha and the softmax normalization fold into one
    per-row output scale. LayerNormed q,k bound the logits (|S| < ~6.6 for
    these inputs), so exp() without max-subtraction is safe.
  * The gate pre-activations (q_n @ [Wsig0|Walp]) are algebraically pushed
    through the LayerNorm: gate_j = rstd*((x@Wqg)_j - mean*csum_j) + badd_j
    with Wqg = Wq @ wg and csum = colsum(wg) precomputed on host. x@Wqg rides
    as two extra columns of the v projection - no extra matmuls.
  * Attention phase computes S^T blocks (keys on partitions) so exp() output
    IS the PV lhsT - no on-chip transposes of the attention matrix. The E
    rowsum rides as a ones-column appended to v.
  * rstd = exp(-0.5*ln(var+eps)), sigmoid/softplus built from exp/ln: the ACT
    engine then needs only the {exp, ln, copy} LUT set.
  * Everything runs in bf16 (same PE rate as f32r on TRN2, half the DMA and
    SBUF): x/W stream in as bf16, qn/kn/E/sigma*v are stored bf16, PSUM
    accumulation stays fp32. fp8 DoubleRow (2x PE) was tried and REJECTED:
    with absmax(out)~0.62 a single e4m3 rounding anywhere (qk / E / v) alone
    produces ~2.2-2.6e-2 max rel err, over the 2e-2 gate; error-compensated
    fp8 needs 2x the streams = parity with bf16.
  * bf16 also fits qnT resident in SBUF (no DRAM roundtrip between phases)
    and makes PE transposes 1.0 cyc/row (vs 1.5 f32r).
  * Input DMAs are issued need-ordered and fine-grained (per contraction
    subtile for the first x tile, per half for weights): with one bulk DMA
    per tensor the first matmul's operands share HBM bandwidth with 13MB of
    later-needed data and PE sat idle ~33us at kernel start.

Sharding: data-parallel over batch B=8 across the 8 NeuronCores (one batch
element per core, weights replicated). No collectives.
"""

import numpy as np


def _ensure_concourse():
    try:
        import concourse.bass  # noqa: F401
        return
    except ImportError:
        pass
    import sys

    for p in ("/opt/trn_rl_repo", "/root/.axon_site/_ro/trn_rl_repo"):
        if p not in sys.path:
            sys.path.insert(0, p)
    import concourse.bass  # noqa: F401


B, T, D = 8, 2048, 768
PT = 128          # partition tile
NT = T // PT      # 16 row tiles
ND = D // PT      # 6 contraction subtiles
NST = 4           # phase-2 super-tiles
STQ = T // NST    # 512 query rows per super-tile
EPS = 1e-5
SCALE = 1.0 / np.sqrt(np.float32(D))
HALF = D // 2     # 384

MM_DT = "bf16"    # matmul dtype: "bf16" (fast) or "f32" (exact, 4x slower)


def _install_ldw_opt():
    """Re-enable walrus's LDWEIGHTS dedup (consecutive matmuls sharing a
    stationary tile skip the reload). Verified bit-identical rel-err on HW."""
    import concourse.bass_utils as bu

    if getattr(bu, "_ldw_opt_installed", False):
        return
    real_run = bu.run_command

    def run_patched(cmd, cwd=None):
        cmd = [
            "--enable-ldw-opt=true" if c == "--enable-ldw-opt=false" else c
            for c in cmd
        ]
        return real_run(cmd, cwd=cwd)

    bu.run_command = run_patched
    bu._ldw_opt_installed = True


def _install_tile_drain_fix():
    """walrus CoreV3 in this image allows only ONE sync-wait per CTRL-class
    (Drain/NoOp) instruction, but TileContext's exit drain accumulates one
    wait per logical processor. Split the waits across single-wait NoOps."""
    import concourse.tile as tile
    import concourse.mybir as mybir
    from concourse.vector_clock import ScopedClock

    if getattr(tile.TileContext, "_drain_fix_installed", False):
        return

    def _patched(self, tick_clock, wait_clock):
        nc = self.nc
        drain_inst = nc.sync.drain()
        wait_clock.add_sem_waits(
            drain_inst.ins, ScopedClock({None: tick_clock.global_clock})
        )
        si = drain_inst.ins.sync_info
        waits = list(si.on_wait or []) if si else []
        if len(waits) > 1:
            si.on_wait = waits[:1]
            for i in range(1, len(waits)):
                nop = nc.sync.nop(nofuse=True, hint="drain_wait_overflow")
                nop.ins.sync_info = mybir.SyncInfo(
                    on_wait=waits[i : i + 1], on_update=[]
                )
        nc.all_engine_barrier()
        assert self.sems is not None
        popped = nc._tile_sem_poison_stack.pop()
        assert popped is self._sem_poison
        nc.clear_and_free_semaphores(list(self.sems.allocated().values()))
        nc.all_engine_barrier()

    tile.TileContext._drain_and_barrier = _patched
    tile.TileContext._drain_fix_installed = True


def _split_excess_waits(nc, mybir, limit=1):
    """walrus CoreV3 here accepts only `limit` sync-waits per instruction.
    Move excess waits onto single-wait NoOps inserted immediately before the
    over-limit instruction on the same engine (waiting earlier on the same
    engine is order-preserving and safe)."""
    blocks = nc.m.functions[0].blocks
    snaps = [(b, list(b.instructions)) for b in blocks]
    plans = []
    for b, insts in snaps:
        plan = []
        for i, inst in enumerate(insts):
            si = inst.sync_info
            waits = list(si.on_wait) if si and si.on_wait else []
            if len(waits) > limit:
                plan.append((i, waits[: len(waits) - limit]))
                si.on_wait = waits[len(waits) - limit :]
        plans.append(plan)
    rebuilt = []
    for (b, insts), plan in zip(snaps, plans):
        plan_by_idx = dict(plan)
        out = []
        for i, inst in enumerate(insts):
            for w in plan_by_idx.get(i, ()):
                nop = nc.engines[inst.engine].nop(nofuse=True, hint="wait_split")
                nop.ins.sync_info = mybir.SyncInfo(on_wait=[w], on_update=[])
                out.append(nop.ins)
            out.append(inst)
        rebuilt.append((b, out))
    # Assign EVERY block (even plan-free ones): nop() auto-appends to the live
    # current bb, so unassigned blocks would keep duplicate stray nops.
    for b, out in rebuilt:
        b.instructions = out


def build_nc(mm_dt_name=MM_DT, ln_gain=False, gate_adds=(0.0, 0.0),
             csum=(0.0, 0.0)):
    """Build the single-core Bass program (SPMD across 8 cores).

    Inputs : xT [NT,PT,ND,PT] (blocked x^T), wq/wk [D, D], wv [D, D+2]
             (= [Wv | Wq@wg/SCALE] with wg = [Wsig[:,0], Walp[:,0]]).
             Optional: qg/qb/kg/kb [PT, D] broadcast LN gain/bias (q's bias
             pre-scaled by SCALE).
    Output : out [T, D]
    gate_adds: per-gate additive consts (bias terms), baked into the program.
    csum   : colsum(wg)/SCALE consts, baked into the program.
    """
    _ensure_concourse()
    import concourse.bass as bass
    import concourse.tile as tile
    import concourse.mybir as mybir
    from concourse.masks import make_identity

    _install_tile_drain_fix()
    # NOTE: walrus's LDW dedup pass (--enable-ldw-opt=true) rejects bf16
    # transpose ldweights in this image; traces show LDWEIGHTS is hidden
    # behind the previous matmul's stream anyway, so it stays disabled.

    f32 = mybir.dt.float32
    mm_dt = {"bf16": mybir.dt.bfloat16, "f32": mybir.dt.float32}[mm_dt_name]
    AF = mybir.ActivationFunctionType
    Alu = mybir.AluOpType

    DV = D + 2  # v projection width incl. gate columns

    nc = bass.Bass()
    xT_d = nc.dram_tensor("xT", [NT, PT, ND, PT], mm_dt, kind="ExternalInput")
    wq_d = nc.dram_tensor("wq", [D, D], mm_dt, kind="ExternalInput")
    wk_d = nc.dram_tensor("wk", [D, D], mm_dt, kind="ExternalInput")
    wv_d = nc.dram_tensor("wv", [D, DV], mm_dt, kind="ExternalInput")
    if ln_gain:
        qg_d = nc.dram_tensor("qg", [PT, D], f32, kind="ExternalInput")
        qb_d = nc.dram_tensor("qb", [PT, D], f32, kind="ExternalInput")
        kg_d = nc.dram_tensor("kg", [PT, D], f32, kind="ExternalInput")
        kb_d = nc.dram_tensor("kb", [PT, D], f32, kind="ExternalInput")
    out_d = nc.dram_tensor("out", [T, D], f32, kind="ExternalOutput")

    with tile.TileContext(nc) as tc:
        with (
            tc.tile_pool(name="persist", bufs=1) as persist,
            tc.tile_pool(name="consts", bufs=1) as consts,
        ):
            ident_f = consts.tile([PT, PT], f32, tag="identf")
            make_identity(nc, ident_f)
            ident = consts.tile([PT, PT], mm_dt, tag="ident")
            nc.vector.tensor_copy(out=ident, in_=ident_f)
            eps_t = consts.tile([PT, 1], f32, tag="eps")
            nc.vector.memset(eps_t, EPS)
            # rstd = exp(-0.5*ln(var+eps) [+ ln(SCALE) for q's fold])
            lnsc_t = consts.tile([PT, 1], f32, tag="lnsc")
            nc.vector.memset(lnsc_t, float(np.log(SCALE)))
            ones16 = consts.tile([PT, NT], f32, tag="ones16")
            nc.vector.memset(ones16, 1.0)

            knT_res = persist.tile([PT, ND, T], mm_dt, tag="knT")
            qnT_res = persist.tile([PT, ND, T], mm_dt, tag="qnT")
            v_res = persist.tile([PT, NT, DV], mm_dt, tag="v")  # +ones cols
            alpha_res = persist.tile([PT, NT], f32, tag="alpha")
            # rowsum rider columns (exact 1.0 in bf16)
            nc.vector.tensor_copy(out=v_res[:, :, D], in_=ones16)
            nc.vector.tensor_copy(out=v_res[:, :, D + 1], in_=ones16)

            if ln_gain:
                qg_t = consts.tile([PT, D], f32, tag="qg")
                qb_t = consts.tile([PT, D], f32, tag="qb")
                kg_t = consts.tile([PT, D], f32, tag="kg")
                kb_t = consts.tile([PT, D], f32, tag="kb")
                nc.sync.dma_start(out=qg_t, in_=qg_d[:, :])
                nc.sync.dma_start(out=qb_t, in_=qb_d[:, :])
                nc.sync.dma_start(out=kg_t, in_=kg_d[:, :])
                nc.sync.dma_start(out=kb_t, in_=kb_d[:, :])

            # ---------------- Phase 1: projections + LN + gates ----------
            with (
                tc.tile_pool(name="weights", bufs=1) as wpool,
                tc.tile_pool(name="ph1", bufs=2) as ph1,
                tc.tile_pool(name="ph1x", bufs=3) as ph1x,
                tc.tile_pool(name="ph1s", bufs=3) as ph1s,
                tc.tile_pool(name="xt_ps", bufs=2, space="PSUM") as xt_ps,
                tc.tile_pool(name="proj_ps", bufs=3, space="PSUM") as proj_ps,
            ):
                wq_t = wpool.tile([PT, ND, D], mm_dt, tag="wq")
                wk_t = wpool.tile([PT, ND, D], mm_dt, tag="wk")
                wv_t = wpool.tile([PT, ND, DV], mm_dt, tag="wv")
                wq_r = wq_d.rearrange("(o p) f -> p o f", p=PT)
                wk_r = wk_d.rearrange("(o p) f -> p o f", p=PT)
                wv_r = wv_d.rearrange("(o p) f -> p o f", p=PT)

                # Need-ordered input streaming: tile 0's dt=0 operands land
                # first, then remaining pieces in consumption order, with
                # x tiles 1-2 prefetched between weight rows. One bulk DMA
                # per tensor would share HBM bandwidth round-robin and stall
                # the first matmul ~30us.
                wsrc = ((wq_t, wq_r, D), (wk_t, wk_r, D), (wv_t, wv_r, DV))
                xT0 = ph1x.tile([PT, ND, PT], mm_dt, tag="xT")
                nc.sync.dma_start(out=xT0[:, 0, :], in_=xT_d[0, :, 0, :])
                for w_t, w_r, cend in wsrc:
                    nc.sync.dma_start(
                        out=w_t[:, 0, 0:HALF], in_=w_r[:, 0, 0:HALF]
                    )
                    nc.sync.dma_start(
                        out=w_t[:, 0, HALF:cend], in_=w_r[:, 0, HALF:cend]
                    )
                nc.sync.dma_start(
                    out=xT0[:, 1:ND, :], in_=xT_d[0, :, 1:ND, :]
                )
                xT1 = ph1x.tile([PT, ND, PT], mm_dt, tag="xT")
                nc.sync.dma_start(out=xT1, in_=xT_d[1])
                for dt in range(1, ND):
                    for w_t, w_r, cend in wsrc:
                        nc.sync.dma_start(
                            out=w_t[:, dt, 0:cend], in_=w_r[:, dt, 0:cend]
                        )
                    if dt == 3:
                        xT2 = ph1x.tile([PT, ND, PT], mm_dt, tag="xT")
                        nc.sync.dma_start(out=xT2, in_=xT_d[2])

                for t in range(NT):
                    r0 = t * PT
                    if t == 0:
                        xT = xT0
                    elif t == 1:
                        xT = xT1
                    elif t == 2:
                        xT = xT2
                    else:
                        xT = ph1x.tile([PT, ND, PT], mm_dt, tag="xT")
                        nc.sync.dma_start(out=xT, in_=xT_d[t])

                    # six psum accumulators (2 tags x 3 bufs rotate across
                    # q/k/v and tiles); dt-outer so the stationary xT tile is
                    # reused by 6 consecutive matmuls
                    qa = proj_ps.tile([PT, HALF], f32, tag="pa")
                    qb = proj_ps.tile([PT, HALF], f32, tag="pb")
                    ka = proj_ps.tile([PT, HALF], f32, tag="pa")
                    kb = proj_ps.tile([PT, HALF], f32, tag="pb")
                    va = proj_ps.tile([PT, HALF], f32, tag="pa")
                    vb = proj_ps.tile([PT, DV - HALF], f32, tag="pb")
                    targets = (
                        (qa, wq_t, 0, HALF), (qb, wq_t, HALF, D),
                        (ka, wk_t, 0, HALF), (kb, wk_t, HALF, D),
                        (va, wv_t, 0, HALF), (vb, wv_t, HALF, DV),
                    )
                    for dt in range(ND):
                        for ps, w_t, c0, c1 in targets:
                            nc.tensor.matmul(
                                ps,
                                lhsT=xT[:, dt, :],
                                rhs=w_t[:, dt, c0:c1],
                                start=(dt == 0),
                                stop=(dt == ND - 1),
                            )

                    # fast PSUM release: stats read psum, ACT copies evacuate
                    # q/k so the next tile's matmuls aren't gated on the
                    # LayerNorm chain (v's psum is held until sigma is ready;
                    # the pa/pb buf rotation gives ~a full tile of slack)
                    gcol = D - HALF  # local index of global col D within vb
                    qsb = ph1.tile([PT, D], mm_dt, tag="qsb")
                    ksb = ph1.tile([PT, D], mm_dt, tag="ksb")
                    gsb = ph1s.tile([PT, 2], f32, tag="gsb")
                    stats_q = ph1s.tile([PT, 2, 6], f32, tag="stq")
                    stats_k = ph1s.tile([PT, 2, 6], f32, tag="stk")
                    # k first throughout: phase 2's S^T needs knT of EVERY
                    # tile before its first matmul, but qnT only of the first
                    # super-tile - k-first shortens the phase boundary wait
                    nc.vector.bn_stats(out=stats_k[:, 0, :], in_=ka)
                    nc.vector.bn_stats(out=stats_k[:, 1, :], in_=kb)
                    nc.vector.bn_stats(out=stats_q[:, 0, :], in_=qa)
                    nc.vector.bn_stats(out=stats_q[:, 1, :], in_=qb)
                    nc.scalar.activation(out=ksb[:, 0:HALF], in_=ka, func=AF.Copy)
                    nc.scalar.activation(out=ksb[:, HALF:D], in_=kb, func=AF.Copy)
                    nc.scalar.activation(out=qsb[:, 0:HALF], in_=qa, func=AF.Copy)
                    nc.scalar.activation(out=qsb[:, HALF:D], in_=qb, func=AF.Copy)
                    nc.vector.tensor_copy(
                        out=gsb, in_=vb[:, gcol : gcol + 2]
                    )

                    # LayerNorm applied in place on the SBUF copies; q folds
                    # SCALE into rstd
                    def layernorm(stats, sb, fold_scale, g_t, b_t, tagn):
                        mv = ph1s.tile([PT, 2], f32, tag=f"mv{tagn}")
                        nc.vector.bn_aggr(out=mv, in_=stats)
                        lnv = ph1s.tile([PT, 1], f32, tag=f"ln{tagn}")
                        nc.scalar.activation(
                            out=lnv, in_=mv[:, 1:2], func=AF.Ln, bias=eps_t
                        )
                        r = ph1s.tile([PT, 1], f32, tag=f"r{tagn}")
                        nc.scalar.activation(
                            out=r, in_=lnv, func=AF.Exp, scale=-0.5,
                            bias=lnsc_t if fold_scale else 0.0,
                        )
                        nc.vector.tensor_scalar(
                            out=sb,
                            in0=sb,
                            scalar1=mv[:, 0:1],
                            scalar2=r,
                            op0=Alu.subtract,
                            op1=Alu.mult,
                        )
                        if g_t is not None:
                            nc.vector.tensor_mul(out=sb, in0=sb, in1=g_t)
                        if b_t is not None:
                            nc.vector.tensor_add(out=sb, in0=sb, in1=b_t)
                        return mv, r

                    # transposes go 3-to-a-PSUM-bank: member 0 starts the
                    # bank (whole 2KB becomes pending-zero), members 1-2
                    # accumulate into their own pending-zero regions. One
                    # wide evac per group (split 2:1 across vector/scalar)
                    # instead of six narrow ones - a single 128x128 evac
                    # (~150ns) can't keep up with the PE's ~53ns transposes
                    def transpose_to(slab, sb, r0):
                        for g in range(ND // 3):
                            tpg = xt_ps.tile([PT, 3, PT], mm_dt, tag="xtp")
                            for i in range(3):
                                dt = g * 3 + i
                                nc.tensor.matmul(
                                    tpg[:, i, :],
                                    lhsT=sb[:, dt * PT : (dt + 1) * PT],
                                    rhs=ident,
                                    is_transpose=True,
                                    start=(i == 0),
                                    stop=(i == 2),
                                    skip_group_check=True,
                                )
                            g0 = g * 3
                            nc.vector.tensor_copy(
                                out=slab[:, g0 : g0 + 2, r0 : r0 + PT],
                                in_=tpg[:, 0:2, :],
                            )
                            nc.scalar.activation(
                                out=slab[:, g0 + 2, r0 : r0 + PT],
                                in_=tpg[:, 2, :], func=AF.Copy,
                            )

                    layernorm(
                        stats_k, ksb, False,
                        kg_t if ln_gain else None, kb_t if ln_gain else None, "k",
                    )
                    transpose_to(knT_res, ksb, r0)

                    mv_q, r_q = layernorm(
                        stats_q, qsb, True,
                        qg_t if ln_gain else None, qb_t if ln_gain else None, "q",
                    )

                    # gates: gate_j = r_q*(raw_j - mean_q*csum_j) + gate_adds_j
                    # (r_q carries 1/SCALE via the fold; csum was pre-divided)
                    sig_t = ph1s.tile([PT, 1], f32, tag="sig")
                    alp_t = ph1s.tile([PT, 1], f32, tag="alp")
                    for j, gout in ((0, sig_t), (1, alp_t)):
                        mc = ph1s.tile([PT, 1], f32, tag=f"mc{j}")
                        nc.vector.tensor_scalar_mul(
                            out=mc, in0=mv_q[:, 0:1], scalar1=float(csum[j])
                        )
                        nc.vector.tensor_scalar(
                            out=gout,
                            in0=gsb[:, j : j + 1],
                            scalar1=mc,
                            scalar2=r_q,
                            op0=Alu.subtract,
                            op1=Alu.mult,
                        )
                        if gate_adds[j] != 0.0:
                            nc.vector.tensor_scalar_add(
                                out=gout, in0=gout, scalar1=float(gate_adds[j])
                            )
                    # sigma = 1/(1+exp(-g0)); alpha = ln(1+exp(g1))
                    nc.scalar.activation(
                        out=sig_t, in_=sig_t, func=AF.Exp, scale=-1.0
                    )
                    nc.vector.tensor_scalar_add(out=sig_t, in0=sig_t, scalar1=1.0)
                    nc.vector.reciprocal(out=sig_t, in_=sig_t)
                    nc.scalar.activation(out=alp_t, in_=alp_t, func=AF.Exp)
                    nc.vector.tensor_scalar_add(out=alp_t, in0=alp_t, scalar1=1.0)
                    nc.scalar.activation(
                        out=alpha_res[:, t : t + 1], in_=alp_t, func=AF.Ln
                    )

                    # v evacuation with the sigma fold, straight to bf16 (one
                    # rounding)
                    nc.vector.tensor_scalar_mul(
                        out=v_res[:, t, 0:HALF], in0=va, scalar1=sig_t
                    )
                    nc.vector.tensor_scalar_mul(
                        out=v_res[:, t, HALF:D], in0=vb[:, 0:gcol], scalar1=sig_t
                    )

                    transpose_to(qnT_res, qsb, r0)

            # ---------------- Phase 2: attention (S^T blocks) -------------
            with (
                tc.tile_pool(name="ph2", bufs=2) as ph2,
                tc.tile_pool(name="ph2s", bufs=3) as ph2s,
                tc.tile_pool(name="s_ps", bufs=3, space="PSUM") as s_ps,
                tc.tile_pool(name="pv_ps", bufs=2, space="PSUM") as pv_ps,
            ):
                for st in range(NST):
                    q0 = st * STQ
                    # S^T blocks: [keys(128) x STQ], exp() lands directly in
                    # the PV lhsT slab
                    ET = ph2.tile([PT, NT, STQ], mm_dt, tag="ET")
                    for kbi in range(NT):
                        sp = s_ps.tile([PT, STQ], f32, tag="s")
                        for dt in range(ND):
                            nc.tensor.matmul(
                                sp,
                                lhsT=knT_res[:, dt, kbi * PT : (kbi + 1) * PT],
                                rhs=qnT_res[:, dt, q0 : q0 + STQ],
                                start=(dt == 0),
                                stop=(dt == ND - 1),
                            )
                        nc.scalar.activation(
                            out=ET[:, kbi, :], in_=sp, func=AF.Exp
                        )

                    for qs in range(NST):
                        t = st * NST + qs
                        r0 = t * PT
                        qsl = slice(qs * PT, (qs + 1) * PT)
                        pp0 = pv_ps.tile([PT, HALF], f32, tag="pv0")
                        pp1 = pv_ps.tile([PT, DV - HALF], f32, tag="pv1")
                        for kbi in range(NT):
                            nc.tensor.matmul(
                                pp0,
                                lhsT=ET[:, kbi, qsl],
                                rhs=v_res[:, kbi, 0:HALF],
                                start=(kbi == 0),
                                stop=(kbi == NT - 1),
                            )
                            nc.tensor.matmul(
                                pp1,
                                lhsT=ET[:, kbi, qsl],
                                rhs=v_res[:, kbi, HALF : DV],
                                start=(kbi == 0),
                                stop=(kbi == NT - 1),
                            )
                        # rowsum rider sits at pp1 col D-HALF; fold alpha and
                        # the softmax normalization into one per-row scale
                        rsc = ph2s.tile([PT, 1], f32, tag="rsc")
                        nc.vector.reciprocal(
                            out=rsc, in_=pp1[:, D - HALF : D - HALF + 1]
                        )
                        rowscale = ph2s.tile([PT, 1], f32, tag="rssc")
                        nc.vector.tensor_mul(
                            out=rowscale, in0=rsc, in1=alpha_res[:, t : t + 1]
                        )
                        o_sb = ph2.tile([PT, D], f32, tag="o")
                        nc.vector.tensor_scalar_mul(
                            out=o_sb[:, 0:HALF], in0=pp0, scalar1=rowscale
                        )
                        nc.vector.tensor_scalar_mul(
                            out=o_sb[:, HALF:D],
                            in0=pp1[:, 0 : D - HALF],
                            scalar1=rowscale,
                        )
                        nc.sync.dma_start(out=out_d[r0 : r0 + PT, :], in_=o_sb)

    _split_excess_waits(nc, mybir)
    return nc


_NC_CACHE = {}


def _get_nc(key):
    if key not in _NC_CACHE:
        _NC_CACHE[key] = build_nc(*key)
    return _NC_CACHE[key]


def make_in_maps(inputs):
    """Host-side prep: per-core input maps + build key."""
    import ml_dtypes

    np_mm = ml_dtypes.bfloat16 if MM_DT == "bf16" else np.float32
    x = np.asarray(inputs["x"], dtype=np.float32)
    Wq = np.asarray(inputs["Wq"], dtype=np.float64)
    Wk = np.asarray(inputs["Wk"], dtype=np.float32)
    Wv = np.asarray(inputs["Wv"], dtype=np.float32)
    qn_g = np.asarray(inputs["qn_g"], dtype=np.float64)
    qn_b = np.asarray(inputs["qn_b"], dtype=np.float64)
    kn_g = np.asarray(inputs["kn_g"], dtype=np.float32)
    kn_b = np.asarray(inputs["kn_b"], dtype=np.float32)
    Wsig = np.asarray(inputs["Wsig"], dtype=np.float64)
    bsig = np.asarray(inputs["bsig"], dtype=np.float64)
    Walp = np.asarray(inputs["Walp"], dtype=np.float64)
    balp = np.asarray(inputs["balp"], dtype=np.float64)

    ln_gain = not (
        np.all(qn_g == 1) and np.all(qn_b == 0)
        and np.all(kn_g == 1) and np.all(kn_b == 0)
    )

    # gate columns: wg = [Wsig[:,0], Walp[:,0]] with LN gain applied; the
    # matmul term rides the v projection as x @ (Wq @ wg) / SCALE (q's rstd
    # carries SCALE). Bias terms (b @ wg + gate bias) are additive consts.
    wg = np.stack([Wsig[:, 0], Walp[:, 0]], axis=1)  # [D, 2] float64
    wg_g = qn_g[:, None] * wg
    Wqg = (Wq @ wg_g) / SCALE                        # [D, 2]
    csum = wg_g.sum(axis=0) / SCALE                  # [2]
    badd = qn_b @ wg                                 # [2]
    gate_adds = (float(badd[0] + bsig[0]), float(badd[1] + balp[0]))

    wv_ext = np.concatenate(
        [Wv.astype(np.float64), Wqg], axis=1
    ).astype(np.float32)  # [D, D+2]

    key = (MM_DT, ln_gain, gate_adds, (float(csum[0]), float(csum[1])))

    base = {
        "wq": np.ascontiguousarray(Wq.astype(np_mm)),
        "wk": np.ascontiguousarray(Wk.astype(np_mm)),
        "wv": np.ascontiguousarray(wv_ext.astype(np_mm)),
    }
    if ln_gain:
        base["qg"] = np.broadcast_to(qn_g.astype(np.float32), (PT, D)).copy()
        base["qb"] = np.broadcast_to(
            (qn_b * SCALE).astype(np.float32), (PT, D)
        ).copy()
        base["kg"] = np.broadcast_to(kn_g, (PT, D)).copy()
        base["kb"] = np.broadcast_to(kn_b, (PT, D)).copy()

    # blocked transpose: xT[t, p, o, f] = x[b, t*PT+f, o*PT+p]
    xTb = np.ascontiguousarray(
        x.reshape(B, NT, PT, ND, PT).transpose(0, 1, 4, 3, 2).astype(np_mm)
    )
    in_maps = [dict(base, xT=xTb[b]) for b in range(B)]
    return in_maps, key


def run(inputs, trace=False, mm_dt=None):
    _ensure_concourse()
    import time
    from concourse.bass_utils import run_bass_kernel_spmd

    in_maps, key = make_in_maps(inputs)
    if mm_dt is not None:
        key = (mm_dt,) + key[1:]
    nc = _get_nc(key)
    res = None
    for attempt in range(3):
        try:
            res = run_bass_kernel_spmd(
                nc, in_maps, core_ids=list(range(B)), trace=trace
            )
            break
        except Exception:
            # transient "accelerator device unrecoverable" wedges heal after
            # a cooldown; retry rather than failing the whole call
            if attempt == 2:
                raise
            time.sleep(75)
    out = np.stack([res.results[b]["out"] for b in range(B)]).astype(np.float32)
    return out, res


def kernel(**inputs) -> np.ndarray:
    out, _ = run(inputs)
    return out


# revision 19
# speedup vs baseline: 1.1646x; 1.0025x over previous
"""Trainium2 Bass kernel for nn_AugmentedAttentionHead.

Per batch element b:
    q = LN(x_b @ Wq); k = LN(x_b @ Wk); v = x_b @ Wv
    S = q k^T / sqrt(D);  P = softmax(S, axis=-1)
    sigma = sigmoid(q @ Wsig + bsig)[:, 0]   (per key)
    alpha = softplus(q @ Walp + balp)        (per query)
    out_b = (P * sigma[None, :] * alpha[:, None]) @ v

Device restructuring:
  * out_b = diag(alpha / rowsum(E)) @ E @ diag(sigma) @ v, E = exp(S/sqrt(D)).
    sigma folds into v rows; alpha and the softmax normalization fold into one
    per-row output scale. LayerNormed q,k bound the logits (|S| < ~6.6 for
    these inputs), so exp() without max-subtraction is safe.
  * The gate pre-activations (q_n @ [Wsig0|Walp]) are algebraically pushed
    through the LayerNorm: gate_j = rstd*((x@Wqg)_j - mean*csum_j) + badd_j
    with Wqg = Wq @ wg and csum = colsum(wg) precomputed on host. x@Wqg rides
    as two extra columns of the v projection - no extra matmuls.
  * Attention phase computes S^T blocks (keys on partitions) so exp() output
    IS the PV lhsT - no on-chip transposes of the attention matrix. The E
    rowsum rides as a ones-column appended to v.
  * rstd = exp(-0.5*ln(var+eps)), sigmoid/softplus built from exp/ln: the ACT
    engine then needs only the {exp, ln, copy} LUT set.
  * Everything runs in bf16 (same PE rate as f32r on TRN2, half the DMA and
    SBUF): x/W stream in as bf16, qn/kn/E/sigma*v are stored bf16, PSUM
    accumulation stays fp32. fp8 DoubleRow (2x PE) was tried and REJECTED:
    with absmax(out)~0.62 a single e4m3 rounding anywhere (qk / E / v) alone
    produces ~2.2-2.6e-2 max rel err, over the 2e-2 gate; error-compensated
    fp8 needs 2x the streams = parity with bf16.
  * bf16 also fits qnT resident in SBUF (no DRAM roundtrip between phases)
    and makes PE transposes 1.0 cyc/row (vs 1.5 f32r).
  * Input DMAs are issued need-ordered and fine-grained (per contraction
    subtile for the first x tile, per half for weights): with one bulk DMA
    per tensor the first matmul's operands share HBM bandwidth with 13MB of
    later-needed data and PE sat idle ~33us at kernel start.

Sharding: data-parallel over batch B=8 across the 8 NeuronCores (one batch
element per core, weights replicated). No collectives.
"""

import numpy as np


def _ensure_concourse():
    try:
        import concourse.bass  # noqa: F401
        return
    except ImportError:
        pass
    import sys

    for p in ("/opt/trn_rl_repo", "/root/.axon_site/_ro/trn_rl_repo"):
        if p not in sys.path:
            sys.path.insert(0, p)
    import concourse.bass  # noqa: F401


B, T, D = 8, 2048, 768
PT = 128          # partition tile
NT = T // PT      # 16 row tiles
ND = D // PT      # 6 contraction subtiles
NST = 4           # phase-2 super-tiles
STQ = T // NST    # 512 query rows per super-tile
EPS = 1e-5
SCALE = 1.0 / np.sqrt(np.float32(D))
HALF = D // 2     # 384

MM_DT = "bf16"    # matmul dtype: "bf16" (fast) or "f32" (exact, 4x slower)


def _install_ldw_opt():
    """Re-enable walrus's LDWEIGHTS dedup (consecutive matmuls sharing a
    stationary tile skip the reload). Verified bit-identical rel-err on HW."""
    import concourse.bass_utils as bu

    if getattr(bu, "_ldw_opt_installed", False):
        return
    real_run = bu.run_command

    def run_patched(cmd, cwd=None):
        cmd = [
            "--enable-ldw-opt=true" if c == "--enable-ldw-opt=false" else c
            for c in cmd
        ]
        return real_run(cmd, cwd=cwd)

    bu.run_command = run_patched
    bu._ldw_opt_installed = True


def _install_tile_drain_fix():
    """walrus CoreV3 in this image allows only ONE sync-wait per CTRL-class
    (Drain/NoOp) instruction, but TileContext's exit drain accumulates one
    wait per logical processor. Split the waits across single-wait NoOps."""
    import concourse.tile as tile
    import concourse.mybir as mybir
    from concourse.vector_clock import ScopedClock

    if getattr(tile.TileContext, "_drain_fix_installed", False):
        return

    def _patched(self, tick_clock, wait_clock):
        nc = self.nc
        drain_inst = nc.sync.drain()
        wait_clock.add_sem_waits(
            drain_inst.ins, ScopedClock({None: tick_clock.global_clock})
        )
        si = drain_inst.ins.sync_info
        waits = list(si.on_wait or []) if si else []
        if len(waits) > 1:
            si.on_wait = waits[:1]
            for i in range(1, len(waits)):
                nop = nc.sync.nop(nofuse=True, hint="drain_wait_overflow")
                nop.ins.sync_info = mybir.SyncInfo(
                    on_wait=waits[i : i + 1], on_update=[]
                )
        nc.all_engine_barrier()
        assert self.sems is not None
        popped = nc._tile_sem_poison_stack.pop()
        assert popped is self._sem_poison
        nc.clear_and_free_semaphores(list(self.sems.allocated().values()))
        nc.all_engine_barrier()

    tile.TileContext._drain_and_barrier = _patched
    tile.TileContext._drain_fix_installed = True


def _split_excess_waits(nc, mybir, limit=1):
    """walrus CoreV3 here accepts only `limit` sync-waits per instruction.
    Move excess waits onto single-wait NoOps inserted immediately before the
    over-limit instruction on the same engine (waiting earlier on the same
    engine is order-preserving and safe)."""
    blocks = nc.m.functions[0].blocks
    snaps = [(b, list(b.instructions)) for b in blocks]
    plans = []
    for b, insts in snaps:
        plan = []
        for i, inst in enumerate(insts):
            si = inst.sync_info
            waits = list(si.on_wait) if si and si.on_wait else []
            if len(waits) > limit:
                plan.append((i, waits[: len(waits) - limit]))
                si.on_wait = waits[len(waits) - limit :]
        plans.append(plan)
    rebuilt = []
    for (b, insts), plan in zip(snaps, plans):
        plan_by_idx = dict(plan)
        out = []
        for i, inst in enumerate(insts):
            for w in plan_by_idx.get(i, ()):
                nop = nc.engines[inst.engine].nop(nofuse=True, hint="wait_split")
                nop.ins.sync_info = mybir.SyncInfo(on_wait=[w], on_update=[])
                out.append(nop.ins)
            out.append(inst)
        rebuilt.append((b, out))
    # Assign EVERY block (even plan-free ones): nop() auto-appends to the live
    # current bb, so unassigned blocks would keep duplicate stray nops.
    for b, out in rebuilt:
        b.instructions = out


def build_nc(mm_dt_name=MM_DT, ln_gain=False, gate_adds=(0.0, 0.0),
             csum=(0.0, 0.0)):
    """Build the single-core Bass program (SPMD across 8 cores).

    Inputs : xT [NT,PT,ND,PT] (blocked x^T), wq/wk [D, D], wv [D, D+2]
             (= [Wv | Wq@wg/SCALE] with wg = [Wsig[:,0], Walp[:,0]]).
             Optional: qg/qb/kg/kb [PT, D] broadcast LN gain/bias (q's bias
             pre-scaled by SCALE).
    Output : out [T, D]
    gate_adds: per-gate additive consts (bias terms), baked into the program.
    csum   : colsum(wg)/SCALE consts, baked into the program.
    """
    _ensure_concourse()
    import concourse.bass as bass
    import concourse.tile as tile
    import concourse.mybir as mybir
    from concourse.masks import make_identity

    _install_tile_drain_fix()
    # NOTE: walrus's LDW dedup pass (--enable-ldw-opt=true) rejects bf16
    # transpose ldweights in this image; traces show LDWEIGHTS is hidden
    # behind the previous matmul's stream anyway, so it stays disabled.

    f32 = mybir.dt.float32
    mm_dt = {"bf16": mybir.dt.bfloat16, "f32": mybir.dt.float32}[mm_dt_name]
    AF = mybir.ActivationFunctionType
    Alu = mybir.AluOpType

    DV = D + 2  # v projection width incl. gate columns

    nc = bass.Bass()
    xT_d = nc.dram_tensor("xT", [NT, PT, ND, PT], mm_dt, kind="ExternalInput")
    wq_d = nc.dram_tensor("wq", [D, D], mm_dt, kind="ExternalInput")
    wk_d = nc.dram_tensor("wk", [D, D], mm_dt, kind="ExternalInput")
    wv_d = nc.dram_tensor("wv", [D, DV], mm_dt, kind="ExternalInput")
    if ln_gain:
        qg_d = nc.dram_tensor("qg", [PT, D], f32, kind="ExternalInput")
        qb_d = nc.dram_tensor("qb", [PT, D], f32, kind="ExternalInput")
        kg_d = nc.dram_tensor("kg", [PT, D], f32, kind="ExternalInput")
        kb_d = nc.dram_tensor("kb", [PT, D], f32, kind="ExternalInput")
    out_d = nc.dram_tensor("out", [T, D], f32, kind="ExternalOutput")

    with tile.TileContext(nc) as tc:
        with (
            tc.tile_pool(name="persist", bufs=1) as persist,
            tc.tile_pool(name="consts", bufs=1) as consts,
        ):
            ident_f = consts.tile([PT, PT], f32, tag="identf")
            make_identity(nc, ident_f)
            ident = consts.tile([PT, PT], mm_dt, tag="ident")
            nc.vector.tensor_copy(out=ident, in_=ident_f)
            eps_t = consts.tile([PT, 1], f32, tag="eps")
            nc.vector.memset(eps_t, EPS)
            # rstd = exp(-0.5*ln(var+eps) [+ ln(SCALE) for q's fold])
            lnsc_t = consts.tile([PT, 1], f32, tag="lnsc")
            nc.vector.memset(lnsc_t, float(np.log(SCALE)))
            ones16 = consts.tile([PT, NT], f32, tag="ones16")
            nc.vector.memset(ones16, 1.0)

            knT_res = persist.tile([PT, ND, T], mm_dt, tag="knT")
            qnT_res = persist.tile([PT, ND, T], mm_dt, tag="qnT")
            v_res = persist.tile([PT, NT, DV], mm_dt, tag="v")  # +ones cols
            alpha_res = persist.tile([PT, NT], f32, tag="alpha")
            # rowsum rider columns (exact 1.0 in bf16)
            nc.vector.tensor_copy(out=v_res[:, :, D], in_=ones16)
            nc.vector.tensor_copy(out=v_res[:, :, D + 1], in_=ones16)

            if ln_gain:
                qg_t = consts.tile([PT, D], f32, tag="qg")
                qb_t = consts.tile([PT, D], f32, tag="qb")
                kg_t = consts.tile([PT, D], f32, tag="kg")
                kb_t = consts.tile([PT, D], f32, tag="kb")
                nc.sync.dma_start(out=qg_t, in_=qg_d[:, :])
                nc.sync.dma_start(out=qb_t, in_=qb_d[:, :])
                nc.sync.dma_start(out=kg_t, in_=kg_d[:, :])
                nc.sync.dma_start(out=kb_t, in_=kb_d[:, :])

            # ---------------- Phase 1: projections + LN + gates ----------
            with (
                tc.tile_pool(name="weights", bufs=1) as wpool,
                tc.tile_pool(name="ph1", bufs=2) as ph1,
                tc.tile_pool(name="ph1x", bufs=3) as ph1x,
                tc.tile_pool(name="ph1s", bufs=3) as ph1s,
                tc.tile_pool(name="xt_ps", bufs=2, space="PSUM") as xt_ps,
                tc.tile_pool(name="proj_ps", bufs=3, space="PSUM") as proj_ps,
            ):
                wq_t = wpool.tile([PT, ND, D], mm_dt, tag="wq")
                wk_t = wpool.tile([PT, ND, D], mm_dt, tag="wk")
                wv_t = wpool.tile([PT, ND, DV], mm_dt, tag="wv")
                wq_r = wq_d.rearrange("(o p) f -> p o f", p=PT)
                wk_r = wk_d.rearrange("(o p) f -> p o f", p=PT)
                wv_r = wv_d.rearrange("(o p) f -> p o f", p=PT)

                # Need-ordered input streaming: tile 0's dt=0 operands land
                # first, then remaining pieces in consumption order, with
                # x tiles 1-2 prefetched between weight rows. One bulk DMA
                # per tensor would share HBM bandwidth round-robin and stall
                # the first matmul ~30us.
                wsrc = ((wq_t, wq_r, D), (wk_t, wk_r, D), (wv_t, wv_r, DV))
                xT0 = ph1x.tile([PT, ND, PT], mm_dt, tag="xT")
                nc.sync.dma_start(out=xT0[:, 0, :], in_=xT_d[0, :, 0, :])
                for w_t, w_r, cend in wsrc:
                    nc.sync.dma_start(
                        out=w_t[:, 0, 0:HALF], in_=w_r[:, 0, 0:HALF]
                    )
                    nc.sync.dma_start(
                        out=w_t[:, 0, HALF:cend], in_=w_r[:, 0, HALF:cend]
                    )
                nc.sync.dma_start(
                    out=xT0[:, 1:ND, :], in_=xT_d[0, :, 1:ND, :]
                )
                xT1 = ph1x.tile([PT, ND, PT], mm_dt, tag="xT")
                nc.sync.dma_start(out=xT1, in_=xT_d[1])
                for dt in range(1, ND):
                    for w_t, w_r, cend in wsrc:
                        nc.sync.dma_start(
                            out=w_t[:, dt, 0:cend], in_=w_r[:, dt, 0:cend]
                        )
                    if dt == 3:
                        xT2 = ph1x.tile([PT, ND, PT], mm_dt, tag="xT")
                        nc.sync.dma_start(out=xT2, in_=xT_d[2])

                # transposes go 3-to-a-PSUM-bank: member 0 starts the bank
                # (whole 2KB becomes pending-zero), members 1-2 accumulate
                # into their own pending-zero regions. One wide evac per
                # group (split 2:1 across vector/scalar) instead of six
                # narrow ones - a single 128x128 evac (~150ns) can't keep up
                # with the PE's ~53ns transposes
                def transpose_to(slab, sb, r0):
                    for g in range(ND // 3):
                        tpg = xt_ps.tile([PT, 3, PT], mm_dt, tag="xtp")
                        for i in range(3):
                            dt = g * 3 + i
                            nc.tensor.matmul(
                                tpg[:, i, :],
                                lhsT=sb[:, dt * PT : (dt + 1) * PT],
                                rhs=ident,
                                is_transpose=True,
                                start=(i == 0),
                                stop=(i == 2),
                                skip_group_check=True,
                            )
                        g0 = g * 3
                        nc.vector.tensor_copy(
                            out=slab[:, g0 : g0 + 2, r0 : r0 + PT],
                            in_=tpg[:, 0:2, :],
                        )
                        nc.scalar.activation(
                            out=slab[:, g0 + 2, r0 : r0 + PT],
                            in_=tpg[:, 2, :], func=AF.Copy,
                        )

                # tile t-1's transposes are emitted AFTER tile t's matmuls
                # (software pipelining): their LayerNorm inputs are then long
                # settled, so the PE never waits on the DVE/ACT LN chain
                pend = None
                for t in range(NT):
                    r0 = t * PT
                    if t == 0:
                        xT = xT0
                    elif t == 1:
                        xT = xT1
                    elif t == 2:
                        xT = xT2
                    else:
                        xT = ph1x.tile([PT, ND, PT], mm_dt, tag="xT")
                        nc.sync.dma_start(out=xT, in_=xT_d[t])

                    # six psum accumulators (2 tags x 3 bufs rotate across
                    # q/k/v and tiles); dt-outer so the stationary xT tile is
                    # reused by 6 consecutive matmuls
                    qa = proj_ps.tile([PT, HALF], f32, tag="pa")
                    qb = proj_ps.tile([PT, HALF], f32, tag="pb")
                    ka = proj_ps.tile([PT, HALF], f32, tag="pa")
                    kb = proj_ps.tile([PT, HALF], f32, tag="pb")
                    va = proj_ps.tile([PT, HALF], f32, tag="pa")
                    vb = proj_ps.tile([PT, DV - HALF], f32, tag="pb")
                    targets = (
                        (qa, wq_t, 0, HALF), (qb, wq_t, HALF, D),
                        (ka, wk_t, 0, HALF), (kb, wk_t, HALF, D),
                        (va, wv_t, 0, HALF), (vb, wv_t, HALF, DV),
                    )
                    for dt in range(ND):
                        for ps, w_t, c0, c1 in targets:
                            nc.tensor.matmul(
                                ps,
                                lhsT=xT[:, dt, :],
                                rhs=w_t[:, dt, c0:c1],
                                start=(dt == 0),
                                stop=(dt == ND - 1),
                            )

                    if pend is not None:
                        pk, pq, pr0 = pend
                        transpose_to(knT_res, pk, pr0)
                        transpose_to(qnT_res, pq, pr0)

                    # fast PSUM release: stats read psum, ACT copies evacuate
                    # q/k so the next tile's matmuls aren't gated on the
                    # LayerNorm chain (v's psum is held until sigma is ready;
                    # the pa/pb buf rotation gives ~a full tile of slack)
                    gcol = D - HALF  # local index of global col D within vb
                    qsb = ph1.tile([PT, D], mm_dt, tag="qsb")
                    ksb = ph1.tile([PT, D], mm_dt, tag="ksb")
                    gsb = ph1s.tile([PT, 2], f32, tag="gsb")
                    stats_q = ph1s.tile([PT, 2, 6], f32, tag="stq")
                    stats_k = ph1s.tile([PT, 2, 6], f32, tag="stk")
                    # k first throughout: phase 2's S^T needs knT of EVERY
                    # tile before its first matmul, but qnT only of the first
                    # super-tile - k-first shortens the phase boundary wait
                    nc.vector.bn_stats(out=stats_k[:, 0, :], in_=ka)
                    nc.vector.bn_stats(out=stats_k[:, 1, :], in_=kb)
                    nc.vector.bn_stats(out=stats_q[:, 0, :], in_=qa)
                    nc.vector.bn_stats(out=stats_q[:, 1, :], in_=qb)
                    nc.scalar.activation(out=ksb[:, 0:HALF], in_=ka, func=AF.Copy)
                    nc.scalar.activation(out=ksb[:, HALF:D], in_=kb, func=AF.Copy)
                    nc.scalar.activation(out=qsb[:, 0:HALF], in_=qa, func=AF.Copy)
                    nc.scalar.activation(out=qsb[:, HALF:D], in_=qb, func=AF.Copy)
                    nc.vector.tensor_copy(
                        out=gsb, in_=vb[:, gcol : gcol + 2]
                    )

                    # LayerNorm applied in place on the SBUF copies; q folds
                    # SCALE into rstd
                    def layernorm(stats, sb, fold_scale, g_t, b_t, tagn):
                        mv = ph1s.tile([PT, 2], f32, tag=f"mv{tagn}")
                        nc.vector.bn_aggr(out=mv, in_=stats)
                        lnv = ph1s.tile([PT, 1], f32, tag=f"ln{tagn}")
                        nc.scalar.activation(
                            out=lnv, in_=mv[:, 1:2], func=AF.Ln, bias=eps_t
                        )
                        r = ph1s.tile([PT, 1], f32, tag=f"r{tagn}")
                        nc.scalar.activation(
                            out=r, in_=lnv, func=AF.Exp, scale=-0.5,
                            bias=lnsc_t if fold_scale else 0.0,
                        )
                        nc.vector.tensor_scalar(
                            out=sb,
                            in0=sb,
                            scalar1=mv[:, 0:1],
                            scalar2=r,
                            op0=Alu.subtract,
                            op1=Alu.mult,
                        )
                        if g_t is not None:
                            nc.vector.tensor_mul(out=sb, in0=sb, in1=g_t)
                        if b_t is not None:
                            nc.vector.tensor_add(out=sb, in0=sb, in1=b_t)
                        return mv, r

                    layernorm(
                        stats_k, ksb, False,
                        kg_t if ln_gain else None, kb_t if ln_gain else None, "k",
                    )
                    mv_q, r_q = layernorm(
                        stats_q, qsb, True,
                        qg_t if ln_gain else None, qb_t if ln_gain else None, "q",
                    )

                    # gates: gate_j = r_q*(raw_j - mean_q*csum_j) + gate_adds_j
                    # (r_q carries 1/SCALE via the fold; csum was pre-divided)
                    sig_t = ph1s.tile([PT, 1], f32, tag="sig")
                    alp_t = ph1s.tile([PT, 1], f32, tag="alp")
                    for j, gout in ((0, sig_t), (1, alp_t)):
                        mc = ph1s.tile([PT, 1], f32, tag=f"mc{j}")
                        nc.vector.tensor_scalar_mul(
                            out=mc, in0=mv_q[:, 0:1], scalar1=float(csum[j])
                        )
                        nc.vector.tensor_scalar(
                            out=gout,
                            in0=gsb[:, j : j + 1],
                            scalar1=mc,
                            scalar2=r_q,
                            op0=Alu.subtract,
                            op1=Alu.mult,
                        )
                        if gate_adds[j] != 0.0:
                            nc.vector.tensor_scalar_add(
                                out=gout, in0=gout, scalar1=float(gate_adds[j])
                            )
                    # sigma = 1/(1+exp(-g0)); alpha = ln(1+exp(g1))
                    nc.scalar.activation(
                        out=sig_t, in_=sig_t, func=AF.Exp, scale=-1.0
                    )
                    nc.vector.tensor_scalar_add(out=sig_t, in0=sig_t, scalar1=1.0)
                    nc.vector.reciprocal(out=sig_t, in_=sig_t)
                    nc.scalar.activation(out=alp_t, in_=alp_t, func=AF.Exp)
                    nc.vector.tensor_scalar_add(out=alp_t, in0=alp_t, scalar1=1.0)
                    nc.scalar.activation(
                        out=alpha_res[:, t : t + 1], in_=alp_t, func=AF.Ln
                    )

                    # v evacuation with the sigma fold, straight to bf16 (one
                    # rounding)
                    nc.vector.tensor_scalar_mul(
                        out=v_res[:, t, 0:HALF], in0=va, scalar1=sig_t
                    )
                    nc.vector.tensor_scalar_mul(
                        out=v_res[:, t, HALF:D], in0=vb[:, 0:gcol], scalar1=sig_t
                    )

                    pend = (ksb, qsb, r0)

                # flush the last tile's transposes (k first: phase 2's S^T
                # needs knT of every tile, qnT only of super-tile 0)
                pk, pq, pr0 = pend
                transpose_to(knT_res, pk, pr0)
                transpose_to(qnT_res, pq, pr0)

            # ---------------- Phase 2: attention (S^T blocks) -------------
            with (
                tc.tile_pool(name="ph2", bufs=2) as ph2,
                tc.tile_pool(name="ph2s", bufs=3) as ph2s,
                tc.tile_pool(name="s_ps", bufs=3, space="PSUM") as s_ps,
                tc.tile_pool(name="pv_ps", bufs=2, space="PSUM") as pv_ps,
            ):
                for st in range(NST):
                    q0 = st * STQ
                    # S^T blocks: [keys(128) x STQ], exp() lands directly in
                    # the PV lhsT slab
                    ET = ph2.tile([PT, NT, STQ], mm_dt, tag="ET")
                    for kbi in range(NT):
                        sp = s_ps.tile([PT, STQ], f32, tag="s")
                        for dt in range(ND):
                            nc.tensor.matmul(
                                sp,
                                lhsT=knT_res[:, dt, kbi * PT : (kbi + 1) * PT],
                                rhs=qnT_res[:, dt, q0 : q0 + STQ],
                                start=(dt == 0),
                                stop=(dt == ND - 1),
                            )
                        nc.scalar.activation(
                            out=ET[:, kbi, :], in_=sp, func=AF.Exp
                        )

                    for qs in range(NST):
                        t = st * NST + qs
                        r0 = t * PT
                        qsl = slice(qs * PT, (qs + 1) * PT)
                        pp0 = pv_ps.tile([PT, HALF], f32, tag="pv0")
                        pp1 = pv_ps.tile([PT, DV - HALF], f32, tag="pv1")
                        for kbi in range(NT):
                            nc.tensor.matmul(
                                pp0,
                                lhsT=ET[:, kbi, qsl],
                                rhs=v_res[:, kbi, 0:HALF],
                                start=(kbi == 0),
                                stop=(kbi == NT - 1),
                            )
                            nc.tensor.matmul(
                                pp1,
                                lhsT=ET[:, kbi, qsl],
                                rhs=v_res[:, kbi, HALF : DV],
                                start=(kbi == 0),
                                stop=(kbi == NT - 1),
                            )
                        # rowsum rider sits at pp1 col D-HALF; fold alpha and
                        # the softmax normalization into one per-row scale
                        rsc = ph2s.tile([PT, 1], f32, tag="rsc")
                        nc.vector.reciprocal(
                            out=rsc, in_=pp1[:, D - HALF : D - HALF + 1]
                        )
                        rowscale = ph2s.tile([PT, 1], f32, tag="rssc")
                        nc.vector.tensor_mul(
                            out=rowscale, in0=rsc, in1=alpha_res[:, t : t + 1]
                        )
                        o_sb = ph2.tile([PT, D], f32, tag="o")
                        nc.vector.tensor_scalar_mul(
                            out=o_sb[:, 0:HALF], in0=pp0, scalar1=rowscale
                        )
                        nc.vector.tensor_scalar_mul(
                            out=o_sb[:, HALF:D],
                            in0=pp1[:, 0 : D - HALF],
                            scalar1=rowscale,
                        )
                        nc.sync.dma_start(out=out_d[r0 : r0 + PT, :], in_=o_sb)

    _split_excess_waits(nc, mybir)
    return nc


_NC_CACHE = {}


def _get_nc(key):
    if key not in _NC_CACHE:
        _NC_CACHE[key] = build_nc(*key)
    return _NC_CACHE[key]


def make_in_maps(inputs):
    """Host-side prep: per-core input maps + build key."""
    import ml_dtypes

    np_mm = ml_dtypes.bfloat16 if MM_DT == "bf16" else np.float32
    x = np.asarray(inputs["x"], dtype=np.float32)
    Wq = np.asarray(inputs["Wq"], dtype=np.float64)
    Wk = np.asarray(inputs["Wk"], dtype=np.float32)
    Wv = np.asarray(inputs["Wv"], dtype=np.float32)
    qn_g = np.asarray(inputs["qn_g"], dtype=np.float64)
    qn_b = np.asarray(inputs["qn_b"], dtype=np.float64)
    kn_g = np.asarray(inputs["kn_g"], dtype=np.float32)
    kn_b = np.asarray(inputs["kn_b"], dtype=np.float32)
    Wsig = np.asarray(inputs["Wsig"], dtype=np.float64)
    bsig = np.asarray(inputs["bsig"], dtype=np.float64)
    Walp = np.asarray(inputs["Walp"], dtype=np.float64)
    balp = np.asarray(inputs["balp"], dtype=np.float64)

    ln_gain = not (
        np.all(qn_g == 1) and np.all(qn_b == 0)
        and np.all(kn_g == 1) and np.all(kn_b == 0)
    )

    # gate columns: wg = [Wsig[:,0], Walp[:,0]] with LN gain applied; the
    # matmul term rides the v projection as x @ (Wq @ wg) / SCALE (q's rstd
    # carries SCALE). Bias terms (b @ wg + gate bias) are additive consts.
    wg = np.stack([Wsig[:, 0], Walp[:, 0]], axis=1)  # [D, 2] float64
    wg_g = qn_g[:, None] * wg
    Wqg = (Wq @ wg_g) / SCALE                        # [D, 2]
    csum = wg_g.sum(axis=0) / SCALE                  # [2]
    badd = qn_b @ wg                                 # [2]
    gate_adds = (float(badd[0] + bsig[0]), float(badd[1] + balp[0]))

    wv_ext = np.concatenate(
        [Wv.astype(np.float64), Wqg], axis=1
    ).astype(np.float32)  # [D, D+2]

    key = (MM_DT, ln_gain, gate_adds, (float(csum[0]), float(csum[1])))

    base = {
        "wq": np.ascontiguousarray(Wq.astype(np_mm)),
        "wk": np.ascontiguousarray(Wk.astype(np_mm)),
        "wv": np.ascontiguousarray(wv_ext.astype(np_mm)),
    }
    if ln_gain:
        base["qg"] = np.broadcast_to(qn_g.astype(np.float32), (PT, D)).copy()
        base["qb"] = np.broadcast_to(
            (qn_b * SCALE).astype(np.float32), (PT, D)
        ).copy()
        base["kg"] = np.broadcast_to(kn_g, (PT, D)).copy()
        base["kb"] = np.broadcast_to(kn_b, (PT, D)).copy()

    # blocked transpose: xT[t, p, o, f] = x[b, t*PT+f, o*PT+p]
    xTb = np.ascontiguousarray(
        x.reshape(B, NT, PT, ND, PT).transpose(0, 1, 4, 3, 2).astype(np_mm)
    )
    in_maps = [dict(base, xT=xTb[b]) for b in range(B)]
    return in_maps, key


def run(inputs, trace=False, mm_dt=None):
    _ensure_concourse()
    import time
    from concourse.bass_utils import run_bass_kernel_spmd

    in_maps, key = make_in_maps(inputs)
    if mm_dt is not None:
        key = (mm_dt,) + key[1:]
    nc = _get_nc(key)
    res = None
    for attempt in range(3):
        try:
            res = run_bass_kernel_spmd(
                nc, in_maps, core_ids=list(range(B)), trace=trace
            )
            break
        except Exception:
            # transient "accelerator device unrecoverable" wedges heal after
            # a cooldown; retry rather than failing the whole call
            if attempt == 2:
                raise
            time.sleep(75)
    out = np.stack([res.results[b]["out"] for b in range(B)]).astype(np.float32)
    return out, res


def kernel(**inputs) -> np.ndarray:
    out, _ = run(inputs)
    return out
